# revision 23
# baseline (speedup 1.0000x reference)
"""Trainium2 Bass kernel for LocalCrossCorrelationWithSmoothnessLoss.

Full inputs in, full output out. Pure data-parallel over batch (B=8 -> 8
NeuronCores); each core computes partial sums for its image; the host
combines them into the three scalar losses.

v2 design (vs baseline): cut elementwise passes + keep PE dense.
  products  sii/sjj via ACT Square (f32->f16), sij via DVE TT.  si/sj maps
            are NOT materialized: pass-1 uses the raw f32r I/J chunks as
            stationary with an f32r unscaled band moving.
  pass 1    fused H-conv + transpose on the PE (per map, 5 chunk-MMs
            accumulate into a 1-bank psum tile) -> T-copy to f16 SBUF
            (ACT/DVE split).
  stage 2   W-conv: band f16 stationary, T f16 moving; a,b land in one
            2-bank psum tile [*,1024]; c,d,e land in one 3-bank psum tile
            [*,1536] (adjacent banks).
  combine   ab_sb = ACT wide copy; P on GPS, (A2|B2) one wide DVE square;
            ONE wide DVE STT (cde_psum - PAB) -> (crossN|IvarN|JvarN)
            bf16; D = GPS TT, R = ACT Rsqrt, q = DVE TT,
            acc += q^2 (ACT Square accum).
  smooth    s^2 (ACT Square accum), lag_w (DVE STT accum), lag_h = free-
            axis DVE STT accum over the host-transposed sT input.

Output per core: [2, 82] partial sums (row 1 = partition-0-masked).
Host assembles the losses in float64.
"""
import sys
import numpy as np

sys.path.insert(0, "/opt/trn_rl_repo")

import ml_dtypes
import bass_rust
import concourse.bass as bass
import concourse.tile as tile
from concourse import mybir
from concourse import bass_utils
from concourse import tile_utils

F32 = mybir.dt.float32
F32R = mybir.dt.float32r
F16 = mybir.dt.float16
BF16 = mybir.dt.bfloat16
ALU = mybir.AluOpType
ACTF = mybir.ActivationFunctionType

H = 1024
W = 1024
PAD = 4
WIN = 81.0
ALPHA = 0.01

# r'-chunks for pass-1 (out range, in range). 512-aligned slices:
# {120,120,120,120,32} x 2.  in = out +- PAD clamped to [0, H].
RCH = []
for _lo in (0, 120, 240, 360, 480, 512, 632, 752, 872, 992):
    _n = 32 if _lo in (480, 992) else 120
    _ilo = max(0, _lo - PAD)
    _ihi = min(H, _lo + _n + PAD)
    RCH.append((_lo, _n, _ilo, _ihi - _ilo))
NRC = len(RCH)

# w'-chunks for stage-2: out w' range + the 128-wide stationary col window.
WCH = []
for _j in range(9):
    _olo = 120 * _j
    _on = min(120, W - _olo)
    _clo = 0 if _j == 0 else (W - 128 if _olo + _on + PAD > W else _olo - PAD)
    WCH.append((_olo, _on, _clo))
NWC = len(WCH)

# map order matters: stage-2 psum layout is a,b -> ab tile; c,d,e (sij,
# sii, sjj) -> cde tile thirds, matching the wide-STT operand layout.
MAPS = ("si", "sj", "sij", "sii", "sjj")

# accumulator columns
ACC_CC = 0          # 18: (j, half)
ACC_S2 = 18         # 16: (ch, tile)
ACC_LW = 34         # 16
ACC_SH = 50         # 32: (ch, tile, half)
NACC = 82

tile_utils.max_sbuf_usage = 207 * 1024

_nc_cache = {}


def _legalize_waits(nc, max_waits=1):
    """walrus accepts only one sync-wait per instruction; split extras
    onto same-engine NoOps placed just before."""
    ctr = 0
    for f in nc.m.functions:
        for bb in f.blocks:
            insts = bb.instructions
            i = 0
            while i < len(insts):
                ins = insts[i]
                si = ins.sync_info
                if si is None:
                    i += 1
                    continue
                w = list(si.on_wait)
                if len(w) <= max_waits:
                    i += 1
                    continue
                extra, keep = w[:-max_waits], w[-max_waits:]
                nops = []
                for j in range(0, len(extra), max_waits):
                    chunk = extra[j:j + max_waits]
                    nop = mybir.InstNoOp(name=f"I-wsplit-{ctr}", ins=[], outs=[])
                    ctr += 1
                    nop.engine = ins.engine
                    nop.sync_info = bass_rust.SyncInfo(on_wait=chunk, on_update=[])
                    nops.append(nop)
                ins.sync_info = bass_rust.SyncInfo(on_wait=keep,
                                                  on_update=list(si.on_update))
                insts[i:i] = nops
                i += len(nops) + 1


def _act_raw(nc, out, in_, func, scale=1.0, accum_out=None):
    """InstActivation without the bass Rsqrt/Reciprocal guard."""
    se = nc.scalar
    bias = nc.const_aps.scalar_like(0.0, in_)
    ins = [se.lower_ap(in_), se.lower_ap(bias),
           mybir.ImmediateValue(dtype=mybir.dt.float32, value=scale),
           mybir.ImmediateValue(dtype=mybir.dt.float32, value=0.0)]
    outs = [se.lower_ap(out)]
    if accum_out is not None:
        outs.append(se.lower_ap(accum_out))
    return se.add_instruction(mybir.InstActivation(
        name=nc.get_next_instruction_name(), func=func, ins=ins, outs=outs))


def _band(klo, kn, olo, on, scale):
    k = np.arange(klo, klo + kn)[:, None]
    m = np.arange(olo, olo + on)[None, :]
    return (np.abs(k - m) <= PAD).astype(np.float32) * scale


def _make_host_consts():
    # bands_f16 tile [128, 544]:
    #   cols   0:120  B0    = |k - m|     <= 4   (unscaled)
    #   cols 120:240  Bmid  = |k - 4 - m| <= 4   (unscaled)
    #   cols 240:304  B8    = |k - 64 - m| <= 4  (unscaled, stage-2 j=8)
    #   cols 304:424  B0s   = B0 * 81
    #   cols 424:544  Bmids = Bmid * 81
    bands = np.zeros((128, 544), dtype=np.float32)
    bands[:, 0:120] = _band(0, 128, 0, 120, 1.0)
    bands[:, 120:240] = _band(0, 128, 4, 120, 1.0)
    bands[:, 240:304] = _band(0, 128, 64, 64, 1.0)
    bands[:, 304:424] = _band(0, 128, 0, 120, WIN)
    bands[:, 424:544] = _band(0, 128, 4, 120, WIN)
    bands_f16 = bands.astype(np.float16)

    # ones [128, 2]: col 0 full, col 1 masks partition 0
    onesp = np.ones((128, 2), dtype=np.float32)
    onesp[0, 1] = 0.0
    return bands_f16, onesp


def _const_map(consts):
    bands_f16, onesp = consts
    return {"bands": bands_f16, "onesp": onesp}


def _band_r16(bands_t, c, scaled):
    """Moving f16 band AP for pass-1 r-chunk c: [r_in rows, out cols]."""
    olo, on, ilo, inn = RCH[c]
    if c == 0:
        base = 304 if scaled else 0
    else:
        base = 424 if scaled else 120
    return bands_t[0:inn, base:base + on]


def _band_w(bands_t, j):
    """Stationary band AP for stage-2 w-chunk j: [128, out cols]."""
    olo, on, clo = WCH[j]
    if j == 0:
        return bands_t[0:128, 0:on]
    if olo - PAD == clo:
        return bands_t[0:128, 120:120 + on]
    return bands_t[0:128, 240:240 + on]


def _build(nc):
    I_d = nc.dram_tensor("I", [H, W], F32R, kind="ExternalInput").ap()
    J_d = nc.dram_tensor("J", [H, W], F32R, kind="ExternalInput").ap()
    s0_d = nc.dram_tensor("s0", [H, W], F32R, kind="ExternalInput").ap()
    s1_d = nc.dram_tensor("s1", [H, W], F32R, kind="ExternalInput").ap()
    sT0_d = nc.dram_tensor("sT0", [W, H], F32R, kind="ExternalInput").ap()
    sT1_d = nc.dram_tensor("sT1", [W, H], F32R, kind="ExternalInput").ap()
    bands_d = nc.dram_tensor("bands", [128, 544], F16,
                             kind="ExternalInput").ap()
    onesp_d = nc.dram_tensor("onesp", [128, 2], F32,
                             kind="ExternalInput").ap()
    part_d = nc.dram_tensor("partials", [2, NACC], F32,
                            kind="ExternalOutput").ap()

    from contextlib import ExitStack
    with tile.TileContext(nc) as tc, ExitStack() as ctx:
        consts = ctx.enter_context(tc.tile_pool(name="consts", bufs=1))
        inp = ctx.enter_context(tc.tile_pool(name="inp", bufs=5))
        xmap = ctx.enter_context(tc.tile_pool(name="xmap", bufs=1))
        tmap = ctx.enter_context(tc.tile_pool(name="tmap", bufs=2))
        ctmp = ctx.enter_context(tc.tile_pool(name="ctmp", bufs=2))
        spool = ctx.enter_context(tc.tile_pool(name="spool", bufs=3))
        sjunk = ctx.enter_context(tc.tile_pool(name="sjunk", bufs=3))
        accp = ctx.enter_context(tc.tile_pool(name="accp", bufs=1))
        psT = ctx.enter_context(tc.tile_pool(name="psT", bufs=3, space="PSUM"))
        psAB = ctx.enter_context(tc.tile_pool(name="psAB", bufs=1,
                                              space="PSUM"))
        psCDE = ctx.enter_context(tc.tile_pool(name="psCDE", bufs=1,
                                               space="PSUM"))

        bands_t = consts.tile([128, 544], F16)
        onesp_t = consts.tile([128, 2], F32)
        nc.sync.dma_start(bands_t[:], bands_d)
        nc.sync.dma_start(onesp_t[:], onesp_d)

        acc = accp.tile([128, NACC], F32)
        nc.vector.memset(acc[:], 0.0)

        # ---------------- emission helpers --------------------------------
        def load_rows(dst, src, r0, n, eng=None):
            eng = eng or nc.sync
            eng.dma_start(dst[0:n, :], src[r0:r0 + n, :])

        s_issued = []
        s_state = [0, 0]  # issued, computed

        def issue_s():
            k = s_state[0]
            if k >= 16:
                return
            s_state[0] += 1
            ch, t = k // 8, k % 8
            s_d = s0_d if ch == 0 else s1_d
            sT_d = sT0_d if ch == 0 else sT1_d
            st = spool.tile([128, W], F32R, tag="s_in")
            load_rows(st, s_d, 128 * t, 128, eng=nc.sync)
            stT = spool.tile([128, H], F32R, tag="sT_in")
            load_rows(stT, sT_d, 128 * t, 128, eng=nc.scalar)
            s_issued.append((st, stT))

        def emit_s_tile():
            """one smoothness tile: s^2 (ACT), lag_w (DVE), lag_h (DVE)."""
            k = s_state[1]
            if k >= 16:
                return
            while s_state[0] < min(16, k + 3):
                issue_s()
            s_state[1] += 1
            st, stT = s_issued[k]
            # s^2 on ACT (Square, accum); out -> junk f16
            o1 = sjunk.tile([128, W], F16, tag="junk")
            nc.scalar.activation(o1[:, :], st[:], ACTF.Square,
                                 accum_out=acc[:, ACC_S2 + k:ACC_S2 + k + 1])
            # lag_w on DVE (fp32 STT accum)
            o2 = sjunk.tile([128, W], F16, tag="junk")
            nc.vector.scalar_tensor_tensor(
                out=o2[:, 0:W - 1], in0=st[:, 1:W], scalar=1.0,
                in1=st[:, 0:W - 1], op0=ALU.mult, op1=ALU.mult,
                accum_out=acc[:, ACC_LW + k:ACC_LW + k + 1])
            # lag_h on DVE from the transposed tile (free-axis lag)
            o3 = sjunk.tile([128, W], F16, tag="junk")
            nc.vector.scalar_tensor_tensor(
                out=o3[:, 0:H - 1], in0=stT[:, 1:H], scalar=1.0,
                in1=stT[:, 0:H - 1], op0=ALU.mult, op1=ALU.mult,
                accum_out=acc[:, ACC_SH + k:ACC_SH + k + 1])

        # ---------------- products (emitted per r-chunk) -------------------
        x_tiles = {}       # (name, c) -> f16 product map tiles
        raw_tiles = {}     # c -> (I_t, J_t) f32r, live for the whole half
        chunk_loaded = set()

        def issue_chunk(c):
            if c in chunk_loaded:
                return
            chunk_loaded.add(c)
            olo, on, ilo, inn = RCH[c]
            I_t = inp.tile([128, W], F32R, tag="I_in",
                           name=f"I_in_{c}")
            J_t = inp.tile([128, W], F32R, tag="J_in",
                           name=f"J_in_{c}")
            load_rows(I_t, I_d, ilo, inn)
            load_rows(J_t, J_d, ilo, inn, eng=nc.scalar)
            raw_tiles[c] = (I_t, J_t)

        def emit_products(c):
            olo, on, ilo, inn = RCH[c]
            issue_chunk(c)
            I_t, J_t = raw_tiles[c]
            for name in ("si", "sj", "sij", "sii", "sjj"):
                xtag = f"X_{name}_c4c9" if c in (4, 9) else f"X_{name}_{c}"
                x_tiles[(name, c)] = xmap.tile([128, W], F16, tag=xtag,
                                               name=f"X_{name}_{c}")
            nc.vector.tensor_copy(x_tiles[("si", c)][0:inn, :], I_t[0:inn, :])
            nc.vector.tensor_copy(x_tiles[("sj", c)][0:inn, :], J_t[0:inn, :])
            nc.scalar.square(x_tiles[("sii", c)][0:inn, :], I_t[0:inn, :])
            nc.scalar.square(x_tiles[("sjj", c)][0:inn, :], J_t[0:inn, :])
            nc.vector.tensor_tensor(out=x_tiles[("sij", c)][0:inn, :],
                                    in0=I_t[0:inn, :], in1=J_t[0:inn, :],
                                    op=ALU.mult)

        for c in range(5):
            issue_chunk(c)
        for c in range(5):
            emit_products(c)

        # ------------- main loop: r-half outer, w-chunks inner -------------
        for hh in range(2):
            cs = list(range(5 * hh, 5 * hh + 5))
            for j, (wolo, won, wclo) in enumerate(WCH):
                wsl = slice(wclo, wclo + 128)
                t_tiles = {}
                # ---- pass 1 + T-copy, per map ----
                for mi, name in enumerate(MAPS):
                    t_tiles[name] = tmap.tile([128, 512], F16,
                                              tag=f"T_{name}",
                                              name=f"T_{name}_{j}_{hh}")
                    pT = psT.tile([128, 512], F32, tag="psT")
                    for c in cs:
                        olo, on, ilo, inn = RCH[c]
                        stat = x_tiles[(name, c)][0:inn, wsl]
                        mov = _band_r16(bands_t, c, mi >= 2)
                        nc.tensor.matmul(
                            pT[0:128, olo - 512 * hh:olo - 512 * hh + on],
                            stat, mov,
                            start=(c == cs[0]), stop=(c == cs[-1]),
                            skip_group_check=True)
                    # T-copy psum -> SBUF f16: balance ACT/DVE
                    on_act = mi < 2 or (mi == 2 and j % 2 == 0)
                    if on_act:
                        nc.scalar.copy(t_tiles[name][:, :], pT[:, :])
                    else:
                        nc.vector.tensor_copy(t_tiles[name][:, :], pT[:, :])

                # ---- stage 2: W-conv into grouped psum tiles ----
                ab_ps = psAB.tile([128, 1024], F32, tag="ab")
                cde_ps = psCDE.tile([128, 1536], F32, tag="cde")
                bw = _band_w(bands_t, j)
                n = won
                nc.tensor.matmul(ab_ps[0:n, 0:512], bw, t_tiles["si"][:, :],
                                 start=True, stop=True)
                nc.tensor.matmul(ab_ps[0:n, 512:1024], bw,
                                 t_tiles["sj"][:, :], start=True, stop=True)
                nc.tensor.matmul(cde_ps[0:n, 0:512], bw, t_tiles["sij"][:, :],
                                 start=True, stop=True)
                nc.tensor.matmul(cde_ps[0:n, 512:1024], bw,
                                 t_tiles["sii"][:, :], start=True, stop=True)
                nc.tensor.matmul(cde_ps[0:n, 1024:1536], bw,
                                 t_tiles["sjj"][:, :], start=True, stop=True)

                # ---- combine ----
                ab_sb = ctmp.tile([128, 1024], BF16, tag="ab_sb")
                nc.scalar.copy(ab_sb[0:n, :], ab_ps[0:n, :])
                a_sb = ab_sb[0:n, 0:512]
                b_sb = ab_sb[0:n, 512:1024]
                pab = ctmp.tile([128, 1536], BF16, tag="pab")
                nc.gpsimd.tensor_tensor(out=pab[0:n, 0:512], in0=a_sb,
                                        in1=b_sb, op=ALU.mult)
                # (A2|B2) in one wide square (src==src runs 1x; one op
                # beats two separate 1x squares)
                nc.vector.tensor_tensor(out=pab[0:n, 512:1536],
                                        in0=ab_sb[0:n, :],
                                        in1=ab_sb[0:n, :], op=ALU.mult)
                cij = ctmp.tile([128, 1536], BF16, tag="cij")
                nc.vector.scalar_tensor_tensor(
                    out=cij[0:n, :], in0=cde_ps[0:n, :], scalar=1.0,
                    in1=pab[0:n, :], op0=ALU.mult, op1=ALU.subtract)
                dd = ctmp.tile([128, 512], BF16, tag="dd")
                nc.gpsimd.tensor_tensor(out=dd[0:n, :],
                                        in0=cij[0:n, 512:1024],
                                        in1=cij[0:n, 1024:1536], op=ALU.mult)
                rs = ctmp.tile([128, 512], BF16, tag="rs")
                _act_raw(nc, rs[0:n, :], dd[0:n, :], ACTF.Rsqrt)
                qj = ctmp.tile([128, 512], BF16, tag="qj")
                nc.gpsimd.tensor_tensor(out=qj[0:n, :], in0=cij[0:n, 0:512],
                                        in1=rs[0:n, :], op=ALU.mult)
                o4 = ctmp.tile([128, 512], F16, tag="o4")
                col = ACC_CC + 2 * j + hh
                nc.scalar.activation(o4[0:n, :], qj[0:n, :], ACTF.Square,
                                     accum_out=acc[0:n, col:col + 1])

                # interleave: next half's products / smoothness tiles
                if hh == 0 and j >= 4:
                    emit_products(j + 1)
                    if j >= 5:
                        emit_s_tile()
                else:
                    emit_s_tile()

        while s_state[1] < 16:
            emit_s_tile()

        # ---------------- final partition reduction ------------------------
        pF = psT.tile([2, NACC], F32, tag="psT")
        nc.tensor.matmul(pF[:], onesp_t[:], acc[:], start=True, stop=True)
        outt = accp.tile([2, NACC], F32, tag="outt")
        nc.scalar.copy(outt[:], pF[:])
        nc.scalar.dma_start(part_d, outt[:])

    return


def _get_nc():
    if "nc" not in _nc_cache:
        nc = bass.Bass("TRN2", target_bir_lowering=False, debug=False)
        _build(nc)
        _legalize_waits(nc)
        _nc_cache["nc"] = nc
    return _nc_cache["nc"]


def _make_in_maps(I, J, s, consts):
    in_maps = []
    for b in range(I.shape[0]):
        m = {
            "I": np.ascontiguousarray(I[b, 0]),
            "J": np.ascontiguousarray(J[b, 0]),
            "s0": np.ascontiguousarray(s[b, 0]),
            "s1": np.ascontiguousarray(s[b, 1]),
            "sT0": np.ascontiguousarray(s[b, 0].T),
            "sT1": np.ascontiguousarray(s[b, 1].T),
        }
        m.update(_const_map(consts))
        in_maps.append(m)
    return in_maps


def kernel(I, J, s, sum_filt):
    B = I.shape[0]
    assert I.shape == (B, 1, H, W) and s.shape == (B, 2, H, W)
    nc = _get_nc()
    consts = _make_host_consts()

    in_maps = _make_in_maps(I, J, s, consts)
    res = bass_utils.run_bass_kernel_spmd(nc, in_maps,
                                          core_ids=list(range(B)))
    parts = np.stack([res.results[b]["partials"] for b in range(B)])
    parts = parts.astype(np.float64)  # [B, 2, NACC]

    s64 = s.astype(np.float64)
    cc_sum = float(parts[:, 0, ACC_CC:ACC_CC + 18].sum())
    lag_w = parts[:, 0, ACC_LW:ACC_LW + 16].sum(axis=1)
    lag_h = parts[:, 0, ACC_SH:ACC_SH + 16].sum(axis=1)
    s2 = parts[:, 0, ACC_S2:ACC_S2 + 16].sum(axis=1)

    # edge corrections per core (both channels folded together)
    e_w = (s64[:, :, :, 0] ** 2).sum(axis=(1, 2)) + \
          (s64[:, :, :, -1] ** 2).sum(axis=(1, 2))
    e_h = (s64[:, :, 0, :] ** 2).sum(axis=(1, 2)) + \
          (s64[:, :, -1, :] ** 2).sum(axis=(1, 2))

    sum_dx2 = (2.0 * s2 - e_w - 2.0 * lag_w).sum()
    sum_dy2 = (2.0 * s2 - e_h - 2.0 * lag_h).sum()
    cnt = B * 2 * H * (W - 1)

    ncc_loss = -cc_sum / (B * H * W)
    smooth = 0.5 * (sum_dx2 / cnt + sum_dy2 / cnt) * ALPHA
    total = ncc_loss + smooth
    return np.array([total, ncc_loss, smooth], dtype=np.float32)


# revision 24
# speedup vs baseline: 1.0309x; 1.0309x over previous
"""Trainium2 Bass kernel for LocalCrossCorrelationWithSmoothnessLoss.

Full inputs in, full output out. Pure data-parallel over batch (B=8 -> 8
NeuronCores); each core computes partial sums for its image; the host
combines them into the three scalar losses.

v2 design (vs baseline): cut elementwise passes + keep PE dense.
  products  sii/sjj via ACT Square (f32->f16), sij via DVE TT.  si/sj maps
            are NOT materialized: pass-1 uses the raw f32r I/J chunks as
            stationary with an f32r unscaled band moving.
  pass 1    fused H-conv + transpose on the PE (per map, 5 chunk-MMs
            accumulate into a 1-bank psum tile) -> T-copy to f16 SBUF
            (ACT/DVE split).
  stage 2   W-conv: band f16 stationary, T f16 moving; a,b land in one
            2-bank psum tile [*,1024]; c,d,e land in one 3-bank psum tile
            [*,1536] (adjacent banks).
  combine   ab_sb = ACT wide copy; P on GPS, (A2|B2) one wide DVE square;
            ONE wide DVE STT (cde_psum - PAB) -> (crossN|IvarN|JvarN)
            bf16; D = GPS TT, R = ACT Rsqrt, q = DVE TT,
            acc += q^2 (ACT Square accum).
  smooth    s^2 (ACT Square accum), lag_w (DVE STT accum), lag_h = free-
            axis DVE STT accum over the host-transposed sT input.

Output per core: [2, 82] partial sums (row 1 = partition-0-masked).
Host assembles the losses in float64.
"""
import sys
import numpy as np

sys.path.insert(0, "/opt/trn_rl_repo")

import ml_dtypes
import bass_rust
import concourse.bass as bass
import concourse.tile as tile
from concourse import mybir
from concourse import bass_utils
from concourse import tile_utils

F32 = mybir.dt.float32
F32R = mybir.dt.float32r
F16 = mybir.dt.float16
BF16 = mybir.dt.bfloat16
ALU = mybir.AluOpType
ACTF = mybir.ActivationFunctionType

H = 1024
W = 1024
PAD = 4
WIN = 81.0
ALPHA = 0.01

# r'-chunks for pass-1 (out range, in range). 512-aligned slices:
# {120,120,120,120,32} x 2.  in = out +- PAD clamped to [0, H].
RCH = []
for _lo in (0, 120, 240, 360, 480, 512, 632, 752, 872, 992):
    _n = 32 if _lo in (480, 992) else 120
    _ilo = max(0, _lo - PAD)
    _ihi = min(H, _lo + _n + PAD)
    RCH.append((_lo, _n, _ilo, _ihi - _ilo))
NRC = len(RCH)

# w'-chunks for stage-2: out w' range + the 128-wide stationary col window.
WCH = []
for _j in range(9):
    _olo = 120 * _j
    _on = min(120, W - _olo)
    _clo = 0 if _j == 0 else (W - 128 if _olo + _on + PAD > W else _olo - PAD)
    WCH.append((_olo, _on, _clo))
NWC = len(WCH)

# map order matters: stage-2 psum layout is a,b -> ab tile; c,d,e (sij,
# sii, sjj) -> cde tile thirds, matching the wide-STT operand layout.
MAPS = ("si", "sj", "sij", "sii", "sjj")

# accumulator columns
ACC_CC = 0          # 18: (j, half)
ACC_S2 = 18         # 16: (ch, tile)
ACC_LW = 34         # 16
ACC_SH = 50         # 32: (ch, tile, half)
NACC = 82

tile_utils.max_sbuf_usage = 207 * 1024

_nc_cache = {}


def _legalize_waits(nc, max_waits=1):
    """walrus accepts only one sync-wait per instruction; split extras
    onto same-engine NoOps placed just before."""
    ctr = 0
    for f in nc.m.functions:
        for bb in f.blocks:
            insts = bb.instructions
            i = 0
            while i < len(insts):
                ins = insts[i]
                si = ins.sync_info
                if si is None:
                    i += 1
                    continue
                w = list(si.on_wait)
                if len(w) <= max_waits:
                    i += 1
                    continue
                extra, keep = w[:-max_waits], w[-max_waits:]
                nops = []
                for j in range(0, len(extra), max_waits):
                    chunk = extra[j:j + max_waits]
                    nop = mybir.InstNoOp(name=f"I-wsplit-{ctr}", ins=[], outs=[])
                    ctr += 1
                    nop.engine = ins.engine
                    nop.sync_info = bass_rust.SyncInfo(on_wait=chunk, on_update=[])
                    nops.append(nop)
                ins.sync_info = bass_rust.SyncInfo(on_wait=keep,
                                                  on_update=list(si.on_update))
                insts[i:i] = nops
                i += len(nops) + 1


def _act_raw(nc, out, in_, func, scale=1.0, accum_out=None):
    """InstActivation without the bass Rsqrt/Reciprocal guard."""
    se = nc.scalar
    bias = nc.const_aps.scalar_like(0.0, in_)
    ins = [se.lower_ap(in_), se.lower_ap(bias),
           mybir.ImmediateValue(dtype=mybir.dt.float32, value=scale),
           mybir.ImmediateValue(dtype=mybir.dt.float32, value=0.0)]
    outs = [se.lower_ap(out)]
    if accum_out is not None:
        outs.append(se.lower_ap(accum_out))
    return se.add_instruction(mybir.InstActivation(
        name=nc.get_next_instruction_name(), func=func, ins=ins, outs=outs))


def _band(klo, kn, olo, on, scale):
    k = np.arange(klo, klo + kn)[:, None]
    m = np.arange(olo, olo + on)[None, :]
    return (np.abs(k - m) <= PAD).astype(np.float32) * scale


def _make_host_consts():
    # bands_f16 tile [128, 544]:
    #   cols   0:120  B0    = |k - m|     <= 4   (unscaled)
    #   cols 120:240  Bmid  = |k - 4 - m| <= 4   (unscaled)
    #   cols 240:304  B8    = |k - 64 - m| <= 4  (unscaled, stage-2 j=8)
    #   cols 304:424  B0s   = B0 * 81
    #   cols 424:544  Bmids = Bmid * 81
    bands = np.zeros((128, 544), dtype=np.float32)
    bands[:, 0:120] = _band(0, 128, 0, 120, 1.0)
    bands[:, 120:240] = _band(0, 128, 4, 120, 1.0)
    bands[:, 240:304] = _band(0, 128, 64, 64, 1.0)
    bands[:, 304:424] = _band(0, 128, 0, 120, WIN)
    bands[:, 424:544] = _band(0, 128, 4, 120, WIN)
    bands_f16 = bands.astype(np.float16)
    bands_f32 = np.ascontiguousarray(bands[:, 0:240])

    # ones [128, 2]: col 0 full, col 1 masks partition 0
    onesp = np.ones((128, 2), dtype=np.float32)
    onesp[0, 1] = 0.0
    return bands_f16, bands_f32, onesp


def _const_map(consts):
    bands_f16, bands_f32, onesp = consts
    return {"bands": bands_f16, "bands32": bands_f32, "onesp": onesp}


def _band_r16(bands_t, c, scaled):
    """Moving f16 band AP for pass-1 r-chunk c: [r_in rows, out cols]."""
    olo, on, ilo, inn = RCH[c]
    if c == 0:
        base = 304 if scaled else 0
    else:
        base = 424 if scaled else 120
    return bands_t[0:inn, base:base + on]


def _band_r32(bands32_t, c):
    olo, on, ilo, inn = RCH[c]
    base = 0 if c == 0 else 120
    return bands32_t[0:inn, base:base + on]


def _band_w(bands_t, j):
    """Stationary band AP for stage-2 w-chunk j: [128, out cols]."""
    olo, on, clo = WCH[j]
    if j == 0:
        return bands_t[0:128, 0:on]
    if olo - PAD == clo:
        return bands_t[0:128, 120:120 + on]
    return bands_t[0:128, 240:240 + on]


def _build(nc):
    I_d = nc.dram_tensor("I", [H, W], F32R, kind="ExternalInput").ap()
    J_d = nc.dram_tensor("J", [H, W], F32R, kind="ExternalInput").ap()
    s0_d = nc.dram_tensor("s0", [H, W], F32R, kind="ExternalInput").ap()
    s1_d = nc.dram_tensor("s1", [H, W], F32R, kind="ExternalInput").ap()
    sT0_d = nc.dram_tensor("sT0", [W, H], F32R, kind="ExternalInput").ap()
    sT1_d = nc.dram_tensor("sT1", [W, H], F32R, kind="ExternalInput").ap()
    bands_d = nc.dram_tensor("bands", [128, 544], F16,
                             kind="ExternalInput").ap()
    bands32_d = nc.dram_tensor("bands32", [128, 240], F32R,
                               kind="ExternalInput").ap()
    onesp_d = nc.dram_tensor("onesp", [128, 2], F32,
                             kind="ExternalInput").ap()
    part_d = nc.dram_tensor("partials", [2, NACC], F32,
                            kind="ExternalOutput").ap()

    from contextlib import ExitStack
    with tile.TileContext(nc) as tc, ExitStack() as ctx:
        consts = ctx.enter_context(tc.tile_pool(name="consts", bufs=1))
        inp = ctx.enter_context(tc.tile_pool(name="inp", bufs=1))
        xmap = ctx.enter_context(tc.tile_pool(name="xmap", bufs=1))
        tmap = ctx.enter_context(tc.tile_pool(name="tmap", bufs=2))
        ctmp = ctx.enter_context(tc.tile_pool(name="ctmp", bufs=2))
        spool = ctx.enter_context(tc.tile_pool(name="spool", bufs=3))
        sjunk = ctx.enter_context(tc.tile_pool(name="sjunk", bufs=3))
        accp = ctx.enter_context(tc.tile_pool(name="accp", bufs=1))
        psT = ctx.enter_context(tc.tile_pool(name="psT", bufs=3, space="PSUM"))
        psAB = ctx.enter_context(tc.tile_pool(name="psAB", bufs=1,
                                              space="PSUM"))
        psCDE = ctx.enter_context(tc.tile_pool(name="psCDE", bufs=1,
                                               space="PSUM"))

        bands_t = consts.tile([128, 544], F16)
        bands32_t = consts.tile([128, 240], F32R)
        onesp_t = consts.tile([128, 2], F32)
        nc.sync.dma_start(bands_t[:], bands_d)
        nc.sync.dma_start(bands32_t[:], bands32_d)
        nc.sync.dma_start(onesp_t[:], onesp_d)

        acc = accp.tile([128, NACC], F32)
        nc.vector.memset(acc[:], 0.0)

        # ---------------- emission helpers --------------------------------
        def load_rows(dst, src, r0, n, eng=None):
            eng = eng or nc.sync
            eng.dma_start(dst[0:n, :], src[r0:r0 + n, :])

        s_issued = []
        s_state = [0, 0]  # issued, computed

        def issue_s():
            k = s_state[0]
            if k >= 16:
                return
            s_state[0] += 1
            ch, t = k // 8, k % 8
            s_d = s0_d if ch == 0 else s1_d
            sT_d = sT0_d if ch == 0 else sT1_d
            st = spool.tile([128, W], F32R, tag="s_in")
            load_rows(st, s_d, 128 * t, 128, eng=nc.sync)
            stT = spool.tile([128, H], F32R, tag="sT_in")
            load_rows(stT, sT_d, 128 * t, 128, eng=nc.scalar)
            s_issued.append((st, stT))

        def emit_s_tile():
            """one smoothness tile: s^2 (ACT), lag_w (DVE), lag_h (DVE)."""
            k = s_state[1]
            if k >= 16:
                return
            while s_state[0] < min(16, k + 3):
                issue_s()
            s_state[1] += 1
            st, stT = s_issued[k]
            # s^2 on ACT (Square, accum); out -> junk f16
            o1 = sjunk.tile([128, W], F16, tag="junk")
            nc.scalar.activation(o1[:, :], st[:], ACTF.Square,
                                 accum_out=acc[:, ACC_S2 + k:ACC_S2 + k + 1])
            # lag_w on DVE (fp32 STT accum)
            o2 = sjunk.tile([128, W], F16, tag="junk")
            nc.vector.scalar_tensor_tensor(
                out=o2[:, 0:W - 1], in0=st[:, 1:W], scalar=1.0,
                in1=st[:, 0:W - 1], op0=ALU.mult, op1=ALU.mult,
                accum_out=acc[:, ACC_LW + k:ACC_LW + k + 1])
            # lag_h on DVE from the transposed tile (free-axis lag)
            o3 = sjunk.tile([128, W], F16, tag="junk")
            nc.vector.scalar_tensor_tensor(
                out=o3[:, 0:H - 1], in0=stT[:, 1:H], scalar=1.0,
                in1=stT[:, 0:H - 1], op0=ALU.mult, op1=ALU.mult,
                accum_out=acc[:, ACC_SH + k:ACC_SH + k + 1])

        # ---------------- products (emitted per r-chunk) -------------------
        x_tiles = {}       # (name, c) -> f16 product map tiles
        raw_tiles = {}     # c -> (I_t, J_t) f32r, live for the whole half
        chunk_loaded = set()

        def issue_chunk(c):
            if c in chunk_loaded:
                return
            chunk_loaded.add(c)
            olo, on, ilo, inn = RCH[c]
            I_t = inp.tile([128, W], F32R, tag=f"I_in_{c}",
                           name=f"I_in_{c}")
            J_t = inp.tile([128, W], F32R, tag=f"J_in_{c}",
                           name=f"J_in_{c}")
            load_rows(I_t, I_d, ilo, inn)
            load_rows(J_t, J_d, ilo, inn, eng=nc.scalar)
            raw_tiles[c] = (I_t, J_t)

        def emit_products(c):
            olo, on, ilo, inn = RCH[c]
            issue_chunk(c)
            I_t, J_t = raw_tiles[c]
            for name in ("sij", "sii", "sjj"):
                xtag = f"X_{name}_c4c9" if c in (4, 9) else f"X_{name}_{c}"
                x_tiles[(name, c)] = xmap.tile([128, W], F16, tag=xtag,
                                               name=f"X_{name}_{c}")
            nc.scalar.square(x_tiles[("sii", c)][0:inn, :], I_t[0:inn, :])
            nc.scalar.square(x_tiles[("sjj", c)][0:inn, :], J_t[0:inn, :])
            nc.vector.tensor_tensor(out=x_tiles[("sij", c)][0:inn, :],
                                    in0=I_t[0:inn, :], in1=J_t[0:inn, :],
                                    op=ALU.mult)

        for c in range(5):
            issue_chunk(c)
        for c in range(5):
            emit_products(c)

        # ------------- main loop: r-half outer, w-chunks inner -------------
        for hh in range(2):
            cs = list(range(5 * hh, 5 * hh + 5))
            for j, (wolo, won, wclo) in enumerate(WCH):
                wsl = slice(wclo, wclo + 128)
                t_tiles = {}
                # ---- pass 1 + T-copy, per map ----
                for mi, name in enumerate(MAPS):
                    t_tiles[name] = tmap.tile([128, 512], F16,
                                              tag=f"T_{name}",
                                              name=f"T_{name}_{j}_{hh}")
                    pT = psT.tile([128, 512], F32, tag="psT")
                    for c in cs:
                        olo, on, ilo, inn = RCH[c]
                        if name == "si":
                            stat = raw_tiles[c][0][0:inn, wsl]
                            mov = _band_r32(bands32_t, c)
                        elif name == "sj":
                            stat = raw_tiles[c][1][0:inn, wsl]
                            mov = _band_r32(bands32_t, c)
                        else:
                            stat = x_tiles[(name, c)][0:inn, wsl]
                            mov = _band_r16(bands_t, c, True)
                        nc.tensor.matmul(
                            pT[0:128, olo - 512 * hh:olo - 512 * hh + on],
                            stat, mov,
                            start=(c == cs[0]), stop=(c == cs[-1]),
                            skip_group_check=True)
                    # T-copy psum -> SBUF f16: balance ACT/DVE
                    on_act = mi < 2 or (mi == 2 and j % 2 == 0)
                    if on_act:
                        nc.scalar.copy(t_tiles[name][:, :], pT[:, :])
                    else:
                        nc.vector.tensor_copy(t_tiles[name][:, :], pT[:, :])

                # ---- stage 2: W-conv into grouped psum tiles ----
                ab_ps = psAB.tile([128, 1024], F32, tag="ab")
                cde_ps = psCDE.tile([128, 1536], F32, tag="cde")
                bw = _band_w(bands_t, j)
                n = won
                nc.tensor.matmul(ab_ps[0:n, 0:512], bw, t_tiles["si"][:, :],
                                 start=True, stop=True)
                nc.tensor.matmul(ab_ps[0:n, 512:1024], bw,
                                 t_tiles["sj"][:, :], start=True, stop=True)
                nc.tensor.matmul(cde_ps[0:n, 0:512], bw, t_tiles["sij"][:, :],
                                 start=True, stop=True)
                nc.tensor.matmul(cde_ps[0:n, 512:1024], bw,
                                 t_tiles["sii"][:, :], start=True, stop=True)
                nc.tensor.matmul(cde_ps[0:n, 1024:1536], bw,
                                 t_tiles["sjj"][:, :], start=True, stop=True)

                # ---- combine ----
                ab_sb = ctmp.tile([128, 1024], BF16, tag="ab_sb")
                nc.scalar.copy(ab_sb[0:n, :], ab_ps[0:n, :])
                a_sb = ab_sb[0:n, 0:512]
                b_sb = ab_sb[0:n, 512:1024]
                pab = ctmp.tile([128, 1536], BF16, tag="pab")
                nc.gpsimd.tensor_tensor(out=pab[0:n, 0:512], in0=a_sb,
                                        in1=b_sb, op=ALU.mult)
                # (A2|B2) in one wide square (src==src runs 1x; one op
                # beats two separate 1x squares)
                nc.vector.tensor_tensor(out=pab[0:n, 512:1536],
                                        in0=ab_sb[0:n, :],
                                        in1=ab_sb[0:n, :], op=ALU.mult)
                cij = ctmp.tile([128, 1536], BF16, tag="cij")
                nc.vector.scalar_tensor_tensor(
                    out=cij[0:n, :], in0=cde_ps[0:n, :], scalar=1.0,
                    in1=pab[0:n, :], op0=ALU.mult, op1=ALU.subtract)
                dd = ctmp.tile([128, 512], BF16, tag="dd")
                nc.gpsimd.tensor_tensor(out=dd[0:n, :],
                                        in0=cij[0:n, 512:1024],
                                        in1=cij[0:n, 1024:1536], op=ALU.mult)
                rs = ctmp.tile([128, 512], BF16, tag="rs")
                _act_raw(nc, rs[0:n, :], dd[0:n, :], ACTF.Rsqrt)
                qj = ctmp.tile([128, 512], BF16, tag="qj")
                nc.gpsimd.tensor_tensor(out=qj[0:n, :], in0=cij[0:n, 0:512],
                                        in1=rs[0:n, :], op=ALU.mult)
                o4 = ctmp.tile([128, 512], F16, tag="o4")
                col = ACC_CC + 2 * j + hh
                nc.scalar.activation(o4[0:n, :], qj[0:n, :], ACTF.Square,
                                     accum_out=acc[0:n, col:col + 1])

                # interleave: next half's products / smoothness tiles
                if hh == 0 and j >= 4:
                    emit_products(j + 1)
                    if j >= 5:
                        emit_s_tile()
                else:
                    emit_s_tile()

        while s_state[1] < 16:
            emit_s_tile()

        # ---------------- final partition reduction ------------------------
        pF = psT.tile([2, NACC], F32, tag="psT")
        nc.tensor.matmul(pF[:], onesp_t[:], acc[:], start=True, stop=True)
        outt = accp.tile([2, NACC], F32, tag="outt")
        nc.scalar.copy(outt[:], pF[:])
        nc.scalar.dma_start(part_d, outt[:])

    return


def _get_nc():
    if "nc" not in _nc_cache:
        nc = bass.Bass("TRN2", target_bir_lowering=False, debug=False)
        _build(nc)
        _legalize_waits(nc)
        _nc_cache["nc"] = nc
    return _nc_cache["nc"]


def _make_in_maps(I, J, s, consts):
    in_maps = []
    for b in range(I.shape[0]):
        m = {
            "I": np.ascontiguousarray(I[b, 0]),
            "J": np.ascontiguousarray(J[b, 0]),
            "s0": np.ascontiguousarray(s[b, 0]),
            "s1": np.ascontiguousarray(s[b, 1]),
            "sT0": np.ascontiguousarray(s[b, 0].T),
            "sT1": np.ascontiguousarray(s[b, 1].T),
        }
        m.update(_const_map(consts))
        in_maps.append(m)
    return in_maps


def kernel(I, J, s, sum_filt):
    B = I.shape[0]
    assert I.shape == (B, 1, H, W) and s.shape == (B, 2, H, W)
    nc = _get_nc()
    consts = _make_host_consts()

    in_maps = _make_in_maps(I, J, s, consts)
    res = bass_utils.run_bass_kernel_spmd(nc, in_maps,
                                          core_ids=list(range(B)))
    parts = np.stack([res.results[b]["partials"] for b in range(B)])
    parts = parts.astype(np.float64)  # [B, 2, NACC]

    s64 = s.astype(np.float64)
    cc_sum = float(parts[:, 0, ACC_CC:ACC_CC + 18].sum())
    lag_w = parts[:, 0, ACC_LW:ACC_LW + 16].sum(axis=1)
    lag_h = parts[:, 0, ACC_SH:ACC_SH + 16].sum(axis=1)
    s2 = parts[:, 0, ACC_S2:ACC_S2 + 16].sum(axis=1)

    # edge corrections per core (both channels folded together)
    e_w = (s64[:, :, :, 0] ** 2).sum(axis=(1, 2)) + \
          (s64[:, :, :, -1] ** 2).sum(axis=(1, 2))
    e_h = (s64[:, :, 0, :] ** 2).sum(axis=(1, 2)) + \
          (s64[:, :, -1, :] ** 2).sum(axis=(1, 2))

    sum_dx2 = (2.0 * s2 - e_w - 2.0 * lag_w).sum()
    sum_dy2 = (2.0 * s2 - e_h - 2.0 * lag_h).sum()
    cnt = B * 2 * H * (W - 1)

    ncc_loss = -cc_sum / (B * H * W)
    smooth = 0.5 * (sum_dx2 / cnt + sum_dy2 / cnt) * ALPHA
    total = ncc_loss + smooth
    return np.array([total, ncc_loss, smooth], dtype=np.float32)


# revision 25
# speedup vs baseline: 1.0394x; 1.0082x over previous
"""Trainium2 Bass kernel for LocalCrossCorrelationWithSmoothnessLoss.

Full inputs in, full output out. Pure data-parallel over batch (B=8 -> 8
NeuronCores); each core computes partial sums for its image; the host
combines them into the three scalar losses.

v2 design (vs baseline): cut elementwise passes + keep PE dense.
  products  sii/sjj via ACT Square (f32->f16), sij via DVE TT.  si/sj maps
            are NOT materialized: pass-1 uses the raw f32r I/J chunks as
            stationary with an f32r unscaled band moving.
  pass 1    fused H-conv + transpose on the PE (per map, 5 chunk-MMs
            accumulate into a 1-bank psum tile) -> T-copy to f16 SBUF
            (ACT/DVE split).
  stage 2   W-conv: band f16 stationary, T f16 moving; a,b land in one
            2-bank psum tile [*,1024]; c,d,e land in one 3-bank psum tile
            [*,1536] (adjacent banks).
  combine   ab_sb = ACT wide copy; P on GPS, (A2|B2) one wide DVE square;
            ONE wide DVE STT (cde_psum - PAB) -> (crossN|IvarN|JvarN)
            bf16; D = GPS TT, R = ACT Rsqrt, q = DVE TT,
            acc += q^2 (ACT Square accum).
  smooth    s^2 (ACT Square accum), lag_w (DVE STT accum), lag_h = free-
            axis DVE STT accum over the host-transposed sT input.

Output per core: [2, 82] partial sums (row 1 = partition-0-masked).
Host assembles the losses in float64.
"""
import sys
import numpy as np

sys.path.insert(0, "/opt/trn_rl_repo")

import ml_dtypes
import bass_rust
import concourse.bass as bass
import concourse.tile as tile
from concourse import mybir
from concourse import bass_utils
from concourse import tile_utils

F32 = mybir.dt.float32
F32R = mybir.dt.float32r
F16 = mybir.dt.float16
BF16 = mybir.dt.bfloat16
ALU = mybir.AluOpType
ACTF = mybir.ActivationFunctionType

H = 1024
W = 1024
PAD = 4
WIN = 81.0
ALPHA = 0.01

# r'-chunks for pass-1 (out range, in range). 512-aligned slices:
# {120,120,120,120,32} x 2.  in = out +- PAD clamped to [0, H].
RCH = []
for _lo in (0, 120, 240, 360, 480, 512, 632, 752, 872, 992):
    _n = 32 if _lo in (480, 992) else 120
    _ilo = max(0, _lo - PAD)
    _ihi = min(H, _lo + _n + PAD)
    RCH.append((_lo, _n, _ilo, _ihi - _ilo))
NRC = len(RCH)

# w'-chunks for stage-2: out w' range + the 128-wide stationary col window.
WCH = []
for _j in range(9):
    _olo = 120 * _j
    _on = min(120, W - _olo)
    _clo = 0 if _j == 0 else (W - 128 if _olo + _on + PAD > W else _olo - PAD)
    WCH.append((_olo, _on, _clo))
NWC = len(WCH)

# map order matters: stage-2 psum layout is a,b -> ab tile; c,d,e (sij,
# sii, sjj) -> cde tile thirds, matching the wide-STT operand layout.
MAPS = ("si", "sj", "sij", "sii", "sjj")

# accumulator columns
ACC_CC = 0          # 18: (j, half)
ACC_S2 = 18         # 16: (ch, tile)
ACC_LW = 34         # 16
ACC_SH = 50         # 32: (ch, tile, half)
NACC = 82

tile_utils.max_sbuf_usage = 207 * 1024

_nc_cache = {}


def _legalize_waits(nc, max_waits=1):
    """walrus accepts only one sync-wait per instruction; split extras
    onto same-engine NoOps placed just before."""
    ctr = 0
    for f in nc.m.functions:
        for bb in f.blocks:
            insts = bb.instructions
            i = 0
            while i < len(insts):
                ins = insts[i]
                si = ins.sync_info
                if si is None:
                    i += 1
                    continue
                w = list(si.on_wait)
                if len(w) <= max_waits:
                    i += 1
                    continue
                extra, keep = w[:-max_waits], w[-max_waits:]
                nops = []
                for j in range(0, len(extra), max_waits):
                    chunk = extra[j:j + max_waits]
                    nop = mybir.InstNoOp(name=f"I-wsplit-{ctr}", ins=[], outs=[])
                    ctr += 1
                    nop.engine = ins.engine
                    nop.sync_info = bass_rust.SyncInfo(on_wait=chunk, on_update=[])
                    nops.append(nop)
                ins.sync_info = bass_rust.SyncInfo(on_wait=keep,
                                                  on_update=list(si.on_update))
                insts[i:i] = nops
                i += len(nops) + 1


def _act_raw(nc, out, in_, func, scale=1.0, accum_out=None):
    """InstActivation without the bass Rsqrt/Reciprocal guard."""
    se = nc.scalar
    bias = nc.const_aps.scalar_like(0.0, in_)
    ins = [se.lower_ap(in_), se.lower_ap(bias),
           mybir.ImmediateValue(dtype=mybir.dt.float32, value=scale),
           mybir.ImmediateValue(dtype=mybir.dt.float32, value=0.0)]
    outs = [se.lower_ap(out)]
    if accum_out is not None:
        outs.append(se.lower_ap(accum_out))
    return se.add_instruction(mybir.InstActivation(
        name=nc.get_next_instruction_name(), func=func, ins=ins, outs=outs))


def _band(klo, kn, olo, on, scale):
    k = np.arange(klo, klo + kn)[:, None]
    m = np.arange(olo, olo + on)[None, :]
    return (np.abs(k - m) <= PAD).astype(np.float32) * scale


def _make_host_consts():
    # bands_f16 tile [128, 544]:
    #   cols   0:120  B0    = |k - m|     <= 4   (unscaled)
    #   cols 120:240  Bmid  = |k - 4 - m| <= 4   (unscaled)
    #   cols 240:304  B8    = |k - 64 - m| <= 4  (unscaled, stage-2 j=8)
    #   cols 304:424  B0s   = B0 * 81
    #   cols 424:544  Bmids = Bmid * 81
    bands = np.zeros((128, 544), dtype=np.float32)
    bands[:, 0:120] = _band(0, 128, 0, 120, 1.0)
    bands[:, 120:240] = _band(0, 128, 4, 120, 1.0)
    bands[:, 240:304] = _band(0, 128, 64, 64, 1.0)
    bands[:, 304:424] = _band(0, 128, 0, 120, WIN)
    bands[:, 424:544] = _band(0, 128, 4, 120, WIN)
    bands_f16 = bands.astype(np.float16)
    bands_f32 = np.ascontiguousarray(bands[:, 0:240])

    # ones [128, 2]: col 0 full, col 1 masks partition 0
    onesp = np.ones((128, 2), dtype=np.float32)
    onesp[0, 1] = 0.0
    return bands_f16, bands_f32, onesp


def _const_map(consts):
    bands_f16, bands_f32, onesp = consts
    return {"bands": bands_f16, "bands32": bands_f32, "onesp": onesp}


def _band_r16(bands_t, c, scaled):
    """Moving f16 band AP for pass-1 r-chunk c: [r_in rows, out cols]."""
    olo, on, ilo, inn = RCH[c]
    if c == 0:
        base = 304 if scaled else 0
    else:
        base = 424 if scaled else 120
    return bands_t[0:inn, base:base + on]


def _band_r32(bands32_t, c):
    olo, on, ilo, inn = RCH[c]
    base = 0 if c == 0 else 120
    return bands32_t[0:inn, base:base + on]


def _band_w(bands_t, j):
    """Stationary band AP for stage-2 w-chunk j: [128, out cols]."""
    olo, on, clo = WCH[j]
    if j == 0:
        return bands_t[0:128, 0:on]
    if olo - PAD == clo:
        return bands_t[0:128, 120:120 + on]
    return bands_t[0:128, 240:240 + on]


def _build(nc):
    I_d = nc.dram_tensor("I", [H, W], F32R, kind="ExternalInput").ap()
    J_d = nc.dram_tensor("J", [H, W], F32R, kind="ExternalInput").ap()
    s0_d = nc.dram_tensor("s0", [H, W], F32R, kind="ExternalInput").ap()
    s1_d = nc.dram_tensor("s1", [H, W], F32R, kind="ExternalInput").ap()
    sT0_d = nc.dram_tensor("sT0", [W, H], F32R, kind="ExternalInput").ap()
    sT1_d = nc.dram_tensor("sT1", [W, H], F32R, kind="ExternalInput").ap()
    bands_d = nc.dram_tensor("bands", [128, 544], F16,
                             kind="ExternalInput").ap()
    bands32_d = nc.dram_tensor("bands32", [128, 240], F32R,
                               kind="ExternalInput").ap()
    onesp_d = nc.dram_tensor("onesp", [128, 2], F32,
                             kind="ExternalInput").ap()
    part_d = nc.dram_tensor("partials", [2, NACC], F32,
                            kind="ExternalOutput").ap()

    from contextlib import ExitStack
    with tile.TileContext(nc) as tc, ExitStack() as ctx:
        consts = ctx.enter_context(tc.tile_pool(name="consts", bufs=1))
        inp = ctx.enter_context(tc.tile_pool(name="inp", bufs=1))
        xmap = ctx.enter_context(tc.tile_pool(name="xmap", bufs=1))
        tmap = ctx.enter_context(tc.tile_pool(name="tmap", bufs=2))
        ctmp = ctx.enter_context(tc.tile_pool(name="ctmp", bufs=2))
        spool = ctx.enter_context(tc.tile_pool(name="spool", bufs=3))
        sjunk = ctx.enter_context(tc.tile_pool(name="sjunk", bufs=3))
        accp = ctx.enter_context(tc.tile_pool(name="accp", bufs=1))
        psT = ctx.enter_context(tc.tile_pool(name="psT", bufs=3, space="PSUM"))
        psAB = ctx.enter_context(tc.tile_pool(name="psAB", bufs=1,
                                              space="PSUM"))
        psCDE = ctx.enter_context(tc.tile_pool(name="psCDE", bufs=1,
                                               space="PSUM"))

        bands_t = consts.tile([128, 544], F16)
        bands32_t = consts.tile([128, 240], F32R)
        onesp_t = consts.tile([128, 2], F32)
        nc.sync.dma_start(bands_t[:], bands_d)
        nc.sync.dma_start(bands32_t[:], bands32_d)
        nc.sync.dma_start(onesp_t[:], onesp_d)

        acc = accp.tile([128, NACC], F32)
        nc.vector.memset(acc[:], 0.0)

        # ---------------- emission helpers --------------------------------
        def load_rows(dst, src, r0, n, eng=None):
            eng = eng or nc.sync
            eng.dma_start(dst[0:n, :], src[r0:r0 + n, :])

        s_issued = []
        s_state = [0, 0, 0]  # issued, lags done, squares done

        def issue_s():
            k = s_state[0]
            if k >= 16:
                return
            s_state[0] += 1
            ch, t = k // 8, k % 8
            s_d = s0_d if ch == 0 else s1_d
            sT_d = sT0_d if ch == 0 else sT1_d
            st = spool.tile([128, W], F32R, tag="s_in")
            load_rows(st, s_d, 128 * t, 128, eng=nc.sync)
            stT = spool.tile([128, H], F32R, tag="sT_in")
            load_rows(stT, sT_d, 128 * t, 128, eng=nc.scalar)
            s_issued.append((st, stT))

        def emit_s_lags():
            """DVE half of one smoothness tile: lag_w + lag_h STT accums.
            Emitted as queue filler ahead of dependent combine DVE ops."""
            k = s_state[1]
            if k >= 16:
                return
            while s_state[0] < min(16, k + 3):
                issue_s()
            s_state[1] += 1
            st, stT = s_issued[k]
            o2 = sjunk.tile([128, W], F16, tag="junk")
            nc.vector.scalar_tensor_tensor(
                out=o2[:, 0:W - 1], in0=st[:, 1:W], scalar=1.0,
                in1=st[:, 0:W - 1], op0=ALU.mult, op1=ALU.mult,
                accum_out=acc[:, ACC_LW + k:ACC_LW + k + 1])
            o3 = sjunk.tile([128, W], F16, tag="junk")
            nc.vector.scalar_tensor_tensor(
                out=o3[:, 0:H - 1], in0=stT[:, 1:H], scalar=1.0,
                in1=stT[:, 0:H - 1], op0=ALU.mult, op1=ALU.mult,
                accum_out=acc[:, ACC_SH + k:ACC_SH + k + 1])

        def emit_s_sq():
            """ACT half: s^2 Square-accum, filler between ab_sb and rs."""
            k = s_state[2]
            if k >= 16 or k >= s_state[1]:
                return
            s_state[2] += 1
            st, stT = s_issued[k]
            o1 = sjunk.tile([128, W], F16, tag="junk")
            nc.scalar.activation(o1[:, :], st[:], ACTF.Square,
                                 accum_out=acc[:, ACC_S2 + k:ACC_S2 + k + 1])

        # ---------------- products (emitted per r-chunk) -------------------
        x_tiles = {}       # (name, c) -> f16 product map tiles
        raw_tiles = {}     # c -> (I_t, J_t) f32r, live for the whole half
        chunk_loaded = set()

        def issue_chunk(c):
            if c in chunk_loaded:
                return
            chunk_loaded.add(c)
            olo, on, ilo, inn = RCH[c]
            I_t = inp.tile([128, W], F32R, tag=f"I_in_{c}",
                           name=f"I_in_{c}")
            J_t = inp.tile([128, W], F32R, tag=f"J_in_{c}",
                           name=f"J_in_{c}")
            load_rows(I_t, I_d, ilo, inn)
            load_rows(J_t, J_d, ilo, inn, eng=nc.scalar)
            raw_tiles[c] = (I_t, J_t)

        def emit_products(c):
            olo, on, ilo, inn = RCH[c]
            issue_chunk(c)
            I_t, J_t = raw_tiles[c]
            for name in ("sij", "sii", "sjj"):
                xtag = f"X_{name}_c4c9" if c in (4, 9) else f"X_{name}_{c}"
                x_tiles[(name, c)] = xmap.tile([128, W], F16, tag=xtag,
                                               name=f"X_{name}_{c}")
            nc.scalar.square(x_tiles[("sii", c)][0:inn, :], I_t[0:inn, :])
            nc.scalar.square(x_tiles[("sjj", c)][0:inn, :], J_t[0:inn, :])
            nc.gpsimd.tensor_tensor(out=x_tiles[("sij", c)][0:inn, :],
                                     in0=I_t[0:inn, :], in1=J_t[0:inn, :],
                                     op=ALU.mult)

        for c in range(5):
            issue_chunk(c)
        for c in range(5):
            emit_products(c)

        # ------------- main loop: r-half outer, w-chunks inner -------------
        for hh in range(2):
            cs = list(range(5 * hh, 5 * hh + 5))
            for j, (wolo, won, wclo) in enumerate(WCH):
                wsl = slice(wclo, wclo + 128)
                t_tiles = {}
                # ---- pass 1 + T-copy, per map ----
                for mi, name in enumerate(MAPS):
                    t_tiles[name] = tmap.tile([128, 512], F16,
                                              tag=f"T_{name}",
                                              name=f"T_{name}_{j}_{hh}")
                    pT = psT.tile([128, 512], F32, tag="psT")
                    for c in cs:
                        olo, on, ilo, inn = RCH[c]
                        if name == "si":
                            stat = raw_tiles[c][0][0:inn, wsl]
                            mov = _band_r32(bands32_t, c)
                        elif name == "sj":
                            stat = raw_tiles[c][1][0:inn, wsl]
                            mov = _band_r32(bands32_t, c)
                        else:
                            stat = x_tiles[(name, c)][0:inn, wsl]
                            mov = _band_r16(bands_t, c, True)
                        nc.tensor.matmul(
                            pT[0:128, olo - 512 * hh:olo - 512 * hh + on],
                            stat, mov,
                            start=(c == cs[0]), stop=(c == cs[-1]),
                            skip_group_check=True)
                    # T-copy psum -> SBUF f16: balance ACT/DVE
                    on_act = mi < 2 or (mi == 2 and j % 2 == 0)
                    if on_act:
                        nc.scalar.copy(t_tiles[name][:, :], pT[:, :])
                    else:
                        nc.vector.tensor_copy(t_tiles[name][:, :], pT[:, :])

                # ---- stage 2: W-conv into grouped psum tiles ----
                ab_ps = psAB.tile([128, 1024], F32, tag="ab")
                cde_ps = psCDE.tile([128, 1536], F32, tag="cde")
                bw = _band_w(bands_t, j)
                n = won
                nc.tensor.matmul(ab_ps[0:n, 0:512], bw, t_tiles["si"][:, :],
                                 start=True, stop=True)
                nc.tensor.matmul(ab_ps[0:n, 512:1024], bw,
                                 t_tiles["sj"][:, :], start=True, stop=True)
                nc.tensor.matmul(cde_ps[0:n, 0:512], bw, t_tiles["sij"][:, :],
                                 start=True, stop=True)
                nc.tensor.matmul(cde_ps[0:n, 512:1024], bw,
                                 t_tiles["sii"][:, :], start=True, stop=True)
                nc.tensor.matmul(cde_ps[0:n, 1024:1536], bw,
                                 t_tiles["sjj"][:, :], start=True, stop=True)

                # ---- iteration filler + combine, ordered per-engine ----
                do_products = (hh == 0 and j >= 4)
                do_s = (not do_products) or j >= 5
                # DVE filler ahead of the ab_sb-dependent combine heads
                if do_s:
                    emit_s_lags()
                # combine heads
                ab_sb = ctmp.tile([128, 1024], BF16, tag="ab_sb")
                nc.scalar.copy(ab_sb[0:n, :], ab_ps[0:n, :])
                a_sb = ab_sb[0:n, 0:512]
                b_sb = ab_sb[0:n, 512:1024]
                pab = ctmp.tile([128, 1536], BF16, tag="pab")
                nc.vector.tensor_tensor(out=pab[0:n, 0:512], in0=a_sb,
                                        in1=b_sb, op=ALU.mult)
                # (A2|B2) in one wide square (src==src runs 1x; one op
                # beats two separate 1x squares)
                nc.vector.tensor_tensor(out=pab[0:n, 512:1536],
                                        in0=ab_sb[0:n, :],
                                        in1=ab_sb[0:n, :], op=ALU.mult)
                cij = ctmp.tile([128, 1536], BF16, tag="cij")
                nc.vector.scalar_tensor_tensor(
                    out=cij[0:n, :], in0=cde_ps[0:n, :], scalar=1.0,
                    in1=pab[0:n, :], op0=ALU.mult, op1=ALU.subtract)
                # ACT/GPS filler while GPS dd waits on the STT
                if do_products:
                    emit_products(j + 1)
                if do_s:
                    emit_s_sq()
                # combine tail
                dd = ctmp.tile([128, 512], BF16, tag="dd")
                nc.gpsimd.tensor_tensor(out=dd[0:n, :],
                                        in0=cij[0:n, 512:1024],
                                        in1=cij[0:n, 1024:1536], op=ALU.mult)
                rs = ctmp.tile([128, 512], BF16, tag="rs")
                _act_raw(nc, rs[0:n, :], dd[0:n, :], ACTF.Rsqrt)
                qj = ctmp.tile([128, 512], BF16, tag="qj")
                nc.gpsimd.tensor_tensor(out=qj[0:n, :], in0=cij[0:n, 0:512],
                                        in1=rs[0:n, :], op=ALU.mult)
                o4 = ctmp.tile([128, 512], F16, tag="o4")
                col = ACC_CC + 2 * j + hh
                nc.scalar.activation(o4[0:n, :], qj[0:n, :], ACTF.Square,
                                     accum_out=acc[0:n, col:col + 1])

        while s_state[1] < 16:
            emit_s_lags()
        while s_state[2] < 16:
            emit_s_sq()

        # ---------------- final partition reduction ------------------------
        pF = psT.tile([2, NACC], F32, tag="psT")
        nc.tensor.matmul(pF[:], onesp_t[:], acc[:], start=True, stop=True)
        outt = accp.tile([2, NACC], F32, tag="outt")
        nc.scalar.copy(outt[:], pF[:])
        nc.scalar.dma_start(part_d, outt[:])

    return


def _get_nc():
    if "nc" not in _nc_cache:
        nc = bass.Bass("TRN2", target_bir_lowering=False, debug=False)
        _build(nc)
        _legalize_waits(nc)
        _nc_cache["nc"] = nc
    return _nc_cache["nc"]


def _make_in_maps(I, J, s, consts):
    in_maps = []
    for b in range(I.shape[0]):
        m = {
            "I": np.ascontiguousarray(I[b, 0]),
            "J": np.ascontiguousarray(J[b, 0]),
            "s0": np.ascontiguousarray(s[b, 0]),
            "s1": np.ascontiguousarray(s[b, 1]),
            "sT0": np.ascontiguousarray(s[b, 0].T),
            "sT1": np.ascontiguousarray(s[b, 1].T),
        }
        m.update(_const_map(consts))
        in_maps.append(m)
    return in_maps


def kernel(I, J, s, sum_filt):
    B = I.shape[0]
    assert I.shape == (B, 1, H, W) and s.shape == (B, 2, H, W)
    nc = _get_nc()
    consts = _make_host_consts()

    in_maps = _make_in_maps(I, J, s, consts)
    res = bass_utils.run_bass_kernel_spmd(nc, in_maps,
                                          core_ids=list(range(B)))
    parts = np.stack([res.results[b]["partials"] for b in range(B)])
    parts = parts.astype(np.float64)  # [B, 2, NACC]

    s64 = s.astype(np.float64)
    cc_sum = float(parts[:, 0, ACC_CC:ACC_CC + 18].sum())
    lag_w = parts[:, 0, ACC_LW:ACC_LW + 16].sum(axis=1)
    lag_h = parts[:, 0, ACC_SH:ACC_SH + 16].sum(axis=1)
    s2 = parts[:, 0, ACC_S2:ACC_S2 + 16].sum(axis=1)

    # edge corrections per core (both channels folded together)
    e_w = (s64[:, :, :, 0] ** 2).sum(axis=(1, 2)) + \
          (s64[:, :, :, -1] ** 2).sum(axis=(1, 2))
    e_h = (s64[:, :, 0, :] ** 2).sum(axis=(1, 2)) + \
          (s64[:, :, -1, :] ** 2).sum(axis=(1, 2))

    sum_dx2 = (2.0 * s2 - e_w - 2.0 * lag_w).sum()
    sum_dy2 = (2.0 * s2 - e_h - 2.0 * lag_h).sum()
    cnt = B * 2 * H * (W - 1)

    ncc_loss = -cc_sum / (B * H * W)
    smooth = 0.5 * (sum_dx2 / cnt + sum_dy2 / cnt) * ALPHA
    total = ncc_loss + smooth
    return np.array([total, ncc_loss, smooth], dtype=np.float32)


# revision 26
# speedup vs baseline: 1.0485x; 1.0087x over previous
"""Trainium2 Bass kernel for LocalCrossCorrelationWithSmoothnessLoss.

Full inputs in, full output out. Pure data-parallel over batch (B=8 -> 8
NeuronCores); each core computes partial sums for its image; the host
combines them into the three scalar losses.

v2 design (vs baseline): cut elementwise passes + keep PE dense.
  products  sii/sjj via ACT Square (f32->f16), sij via DVE TT.  si/sj maps
            are NOT materialized: pass-1 uses the raw f32r I/J chunks as
            stationary with an f32r unscaled band moving.
  pass 1    fused H-conv + transpose on the PE (per map, 5 chunk-MMs
            accumulate into a 1-bank psum tile) -> T-copy to f16 SBUF
            (ACT/DVE split).
  stage 2   W-conv: band f16 stationary, T f16 moving; a,b land in one
            2-bank psum tile [*,1024]; c,d,e land in one 3-bank psum tile
            [*,1536] (adjacent banks).
  combine   ab_sb = ACT wide copy; P on GPS, (A2|B2) one wide DVE square;
            ONE wide DVE STT (cde_psum - PAB) -> (crossN|IvarN|JvarN)
            bf16; D = GPS TT, R = ACT Rsqrt, q = DVE TT,
            acc += q^2 (ACT Square accum).
  smooth    s^2 (ACT Square accum), lag_w (DVE STT accum), lag_h = free-
            axis DVE STT accum over the host-transposed sT input.

Output per core: [2, 82] partial sums (row 1 = partition-0-masked).
Host assembles the losses in float64.
"""
import sys
import numpy as np

sys.path.insert(0, "/opt/trn_rl_repo")

import ml_dtypes
import bass_rust
import concourse.bass as bass
import concourse.tile as tile
from concourse import mybir
from concourse import bass_utils
from concourse import tile_utils

F32 = mybir.dt.float32
F32R = mybir.dt.float32r
F16 = mybir.dt.float16
BF16 = mybir.dt.bfloat16
ALU = mybir.AluOpType
ACTF = mybir.ActivationFunctionType

H = 1024
W = 1024
PAD = 4
WIN = 81.0
ALPHA = 0.01

# r'-chunks for pass-1 (out range, in range). 512-aligned slices:
# {120,120,120,120,32} x 2.  in = out +- PAD clamped to [0, H].
RCH = []
for _lo in (0, 120, 240, 360, 480, 512, 632, 752, 872, 992):
    _n = 32 if _lo in (480, 992) else 120
    _ilo = max(0, _lo - PAD)
    _ihi = min(H, _lo + _n + PAD)
    RCH.append((_lo, _n, _ilo, _ihi - _ilo))
NRC = len(RCH)

# w'-chunks for stage-2: out w' range + the 128-wide stationary col window.
WCH = []
for _j in range(9):
    _olo = 120 * _j
    _on = min(120, W - _olo)
    _clo = 0 if _j == 0 else (W - 128 if _olo + _on + PAD > W else _olo - PAD)
    WCH.append((_olo, _on, _clo))
NWC = len(WCH)

# map order matters: stage-2 psum layout is a,b -> ab tile; c,d,e (sij,
# sii, sjj) -> cde tile thirds, matching the wide-STT operand layout.
MAPS = ("si", "sj", "sij", "sii", "sjj")

# accumulator columns
ACC_CC = 0          # 18: (j, half)
ACC_S2 = 18         # 16: (ch, tile)
ACC_LW = 34         # 16
ACC_SH = 50         # 32: (ch, tile, half)
NACC = 82

tile_utils.max_sbuf_usage = 207 * 1024

_nc_cache = {}


def _legalize_waits(nc, max_waits=1):
    """walrus accepts only one sync-wait per instruction; split extras
    onto same-engine NoOps placed just before."""
    ctr = 0
    for f in nc.m.functions:
        for bb in f.blocks:
            insts = bb.instructions
            i = 0
            while i < len(insts):
                ins = insts[i]
                si = ins.sync_info
                if si is None:
                    i += 1
                    continue
                w = list(si.on_wait)
                if len(w) <= max_waits:
                    i += 1
                    continue
                extra, keep = w[:-max_waits], w[-max_waits:]
                nops = []
                for j in range(0, len(extra), max_waits):
                    chunk = extra[j:j + max_waits]
                    nop = mybir.InstNoOp(name=f"I-wsplit-{ctr}", ins=[], outs=[])
                    ctr += 1
                    nop.engine = ins.engine
                    nop.sync_info = bass_rust.SyncInfo(on_wait=chunk, on_update=[])
                    nops.append(nop)
                ins.sync_info = bass_rust.SyncInfo(on_wait=keep,
                                                  on_update=list(si.on_update))
                insts[i:i] = nops
                i += len(nops) + 1


def _act_raw(nc, out, in_, func, scale=1.0, accum_out=None):
    """InstActivation without the bass Rsqrt/Reciprocal guard."""
    se = nc.scalar
    bias = nc.const_aps.scalar_like(0.0, in_)
    ins = [se.lower_ap(in_), se.lower_ap(bias),
           mybir.ImmediateValue(dtype=mybir.dt.float32, value=scale),
           mybir.ImmediateValue(dtype=mybir.dt.float32, value=0.0)]
    outs = [se.lower_ap(out)]
    if accum_out is not None:
        outs.append(se.lower_ap(accum_out))
    return se.add_instruction(mybir.InstActivation(
        name=nc.get_next_instruction_name(), func=func, ins=ins, outs=outs))


def _band(klo, kn, olo, on, scale):
    k = np.arange(klo, klo + kn)[:, None]
    m = np.arange(olo, olo + on)[None, :]
    return (np.abs(k - m) <= PAD).astype(np.float32) * scale


def _make_host_consts():
    # bands_f16 tile [128, 544]:
    #   cols   0:120  B0    = |k - m|     <= 4   (unscaled)
    #   cols 120:240  Bmid  = |k - 4 - m| <= 4   (unscaled)
    #   cols 240:304  B8    = |k - 64 - m| <= 4  (unscaled, stage-2 j=8)
    #   cols 304:424  B0s   = B0 * 81
    #   cols 424:544  Bmids = Bmid * 81
    bands = np.zeros((128, 544), dtype=np.float32)
    bands[:, 0:120] = _band(0, 128, 0, 120, 1.0)
    bands[:, 120:240] = _band(0, 128, 4, 120, 1.0)
    bands[:, 240:304] = _band(0, 128, 64, 64, 1.0)
    bands[:, 304:424] = _band(0, 128, 0, 120, WIN)
    bands[:, 424:544] = _band(0, 128, 4, 120, WIN)
    bands_f16 = bands.astype(np.float16)
    bands_bf = bands[:, 0:240].astype(ml_dtypes.bfloat16)

    # ones [128, 2]: col 0 full, col 1 masks partition 0
    onesp = np.ones((128, 2), dtype=np.float32)
    onesp[0, 1] = 0.0
    return bands_f16, bands_bf, onesp


def _const_map(consts):
    bands_f16, bands_bf, onesp = consts
    return {"bands": bands_f16, "bandsbf": bands_bf, "onesp": onesp}


def _band_r16(bands_t, c, scaled):
    """Moving f16 band AP for pass-1 r-chunk c: [r_in rows, out cols]."""
    olo, on, ilo, inn = RCH[c]
    if c == 0:
        base = 304 if scaled else 0
    else:
        base = 424 if scaled else 120
    return bands_t[0:inn, base:base + on]


def _band_rbf(bandsbf_t, c):
    olo, on, ilo, inn = RCH[c]
    base = 0 if c == 0 else 120
    return bandsbf_t[0:inn, base:base + on]


def _band_w(bands_t, j):
    """Stationary band AP for stage-2 w-chunk j: [128, out cols]."""
    olo, on, clo = WCH[j]
    if j == 0:
        return bands_t[0:128, 0:on]
    if olo - PAD == clo:
        return bands_t[0:128, 120:120 + on]
    return bands_t[0:128, 240:240 + on]


def _build(nc):
    I_d = nc.dram_tensor("I", [H, W], F32R, kind="ExternalInput").ap()
    J_d = nc.dram_tensor("J", [H, W], F32R, kind="ExternalInput").ap()
    s0_d = nc.dram_tensor("s0", [H, W], F32R, kind="ExternalInput").ap()
    s1_d = nc.dram_tensor("s1", [H, W], F32R, kind="ExternalInput").ap()
    sT0_d = nc.dram_tensor("sT0", [W, H], F32R, kind="ExternalInput").ap()
    sT1_d = nc.dram_tensor("sT1", [W, H], F32R, kind="ExternalInput").ap()
    bands_d = nc.dram_tensor("bands", [128, 544], F16,
                             kind="ExternalInput").ap()
    bandsbf_d = nc.dram_tensor("bandsbf", [128, 240], BF16,
                               kind="ExternalInput").ap()
    onesp_d = nc.dram_tensor("onesp", [128, 2], F32,
                             kind="ExternalInput").ap()
    part_d = nc.dram_tensor("partials", [2, NACC], F32,
                            kind="ExternalOutput").ap()

    from contextlib import ExitStack
    with tile.TileContext(nc) as tc, ExitStack() as ctx:
        consts = ctx.enter_context(tc.tile_pool(name="consts", bufs=1))
        inp = ctx.enter_context(tc.tile_pool(name="inp", bufs=1))
        xmap = ctx.enter_context(tc.tile_pool(name="xmap", bufs=1))
        tmap = ctx.enter_context(tc.tile_pool(name="tmap", bufs=2))
        ctmp = ctx.enter_context(tc.tile_pool(name="ctmp", bufs=2))
        spool = ctx.enter_context(tc.tile_pool(name="spool", bufs=3))
        sjunk = ctx.enter_context(tc.tile_pool(name="sjunk", bufs=3))
        accp = ctx.enter_context(tc.tile_pool(name="accp", bufs=1))
        psT = ctx.enter_context(tc.tile_pool(name="psT", bufs=3, space="PSUM"))
        psAB = ctx.enter_context(tc.tile_pool(name="psAB", bufs=1,
                                              space="PSUM"))
        psCDE = ctx.enter_context(tc.tile_pool(name="psCDE", bufs=1,
                                               space="PSUM"))

        bands_t = consts.tile([128, 544], F16)
        bandsbf_t = consts.tile([128, 240], BF16)
        onesp_t = consts.tile([128, 2], F32)
        nc.sync.dma_start(bands_t[:], bands_d)
        nc.sync.dma_start(bandsbf_t[:], bandsbf_d)
        nc.sync.dma_start(onesp_t[:], onesp_d)

        acc = accp.tile([128, NACC], F32)
        nc.vector.memset(acc[:], 0.0)

        # PE warm-up: ~4us of dummy matmuls while input DMA is in flight,
        # so HAM un-throttles (K=8/8) before the real work arrives.
        warm_ps = psT.tile([128, 512], F32, tag="psT")
        for wk in range(9):
            nc.tensor.matmul(warm_ps[0:120, 0:512], bands_t[0:128, 0:120],
                             bands_t[0:128, 0:512], start=(wk == 0),
                             stop=(wk == 8), skip_group_check=True)

        # ---------------- emission helpers --------------------------------
        def load_rows(dst, src, r0, n, eng=None):
            eng = eng or nc.sync
            eng.dma_start(dst[0:n, :], src[r0:r0 + n, :])

        s_issued = []
        s_state = [0, 0, 0]  # issued, lags done, squares done

        def issue_s():
            k = s_state[0]
            if k >= 16:
                return
            s_state[0] += 1
            ch, t = k // 8, k % 8
            s_d = s0_d if ch == 0 else s1_d
            sT_d = sT0_d if ch == 0 else sT1_d
            comb = spool.tile([128, W + H], F32R, tag="s_in")
            nc.sync.dma_start(comb[0:128, 0:W], s_d[128 * t:128 * t + 128, :])
            nc.scalar.dma_start(comb[0:128, W:W + H],
                                sT_d[128 * t:128 * t + 128, :])
            s_issued.append(comb)

        def emit_s_lags():
            """one fused lag pass: (lag_w + lag_h + seam) STT accum over the
            combined s|sT tile; the host subtracts the seam term."""
            k = s_state[1]
            if k >= 16:
                return
            while s_state[0] < min(16, k + 3):
                issue_s()
            s_state[1] += 1
            comb = s_issued[k]
            o2 = sjunk.tile([128, W + H], F16, tag="junk")
            nc.vector.scalar_tensor_tensor(
                out=o2[:, 0:W + H - 1], in0=comb[:, 1:W + H], scalar=1.0,
                in1=comb[:, 0:W + H - 1], op0=ALU.mult, op1=ALU.mult,
                accum_out=acc[:, ACC_LW + k:ACC_LW + k + 1])

        def emit_s_sq():
            """ACT half: s^2 Square-accum, filler between ab_sb and rs."""
            k = s_state[2]
            if k >= 16 or k >= s_state[1]:
                return
            s_state[2] += 1
            comb = s_issued[k]
            o1 = sjunk.tile([128, W + H], F16, tag="junk")
            nc.scalar.activation(o1[:, 0:W], comb[:, 0:W], ACTF.Square,
                                 accum_out=acc[:, ACC_S2 + k:ACC_S2 + k + 1])

        # ---------------- products (emitted per r-chunk) -------------------
        x_tiles = {}       # (name, c) -> f16 product map tiles
        raw_tiles = {}     # c -> (I_t, J_t) f32r, live for the whole half
        chunk_loaded = set()

        def issue_chunk(c):
            if c in chunk_loaded:
                return
            chunk_loaded.add(c)
            olo, on, ilo, inn = RCH[c]
            I_t = inp.tile([128, W], F32R, tag=f"I_in_{c}",
                           name=f"I_in_{c}")
            J_t = inp.tile([128, W], F32R, tag=f"J_in_{c}",
                           name=f"J_in_{c}")
            load_rows(I_t, I_d, ilo, inn)
            load_rows(J_t, J_d, ilo, inn, eng=nc.scalar)
            raw_tiles[c] = (I_t, J_t)

        def emit_products(c):
            olo, on, ilo, inn = RCH[c]
            issue_chunk(c)
            I_t, J_t = raw_tiles[c]
            for name in ("sij", "sii", "sjj"):
                xtag = f"X_{name}_c4c9" if c in (4, 9) else f"X_{name}_{c}"
                x_tiles[(name, c)] = xmap.tile([128, W], F16, tag=xtag,
                                               name=f"X_{name}_{c}")
            nc.scalar.square(x_tiles[("sii", c)][0:inn, :], I_t[0:inn, :])
            nc.scalar.square(x_tiles[("sjj", c)][0:inn, :], J_t[0:inn, :])
            nc.gpsimd.tensor_tensor(out=x_tiles[("sij", c)][0:inn, :],
                                     in0=I_t[0:inn, :], in1=J_t[0:inn, :],
                                     op=ALU.mult)

        for c in range(5):
            issue_chunk(c)
        for c in range(5):
            emit_products(c)

        # ------------- main loop: r-half outer, w-chunks inner -------------
        for hh in range(2):
            cs = list(range(5 * hh, 5 * hh + 5))
            for j, (wolo, won, wclo) in enumerate(WCH):
                wsl = slice(wclo, wclo + 128)
                t_tiles = {}
                # ---- pass 1 + T-copy, per map ----
                for mi, name in enumerate(MAPS):
                    t_tiles[name] = tmap.tile([128, 512], F16,
                                              tag=f"T_{name}",
                                              name=f"T_{name}_{j}_{hh}")
                    pT = psT.tile([128, 512], F32, tag="psT")
                    bsl = slice(2 * wclo + 1, 2 * (wclo + 128), 2)
                    for c in cs:
                        olo, on, ilo, inn = RCH[c]
                        if name == "si":
                            stat = raw_tiles[c][0].bitcast(BF16)[0:inn, bsl]
                            mov = _band_rbf(bandsbf_t, c)
                        elif name == "sj":
                            stat = raw_tiles[c][1].bitcast(BF16)[0:inn, bsl]
                            mov = _band_rbf(bandsbf_t, c)
                        else:
                            stat = x_tiles[(name, c)][0:inn, wsl]
                            mov = _band_r16(bands_t, c, True)
                        nc.tensor.matmul(
                            pT[0:128, olo - 512 * hh:olo - 512 * hh + on],
                            stat, mov,
                            start=(c == cs[0]), stop=(c == cs[-1]),
                            skip_group_check=True)
                    # T-copy psum -> SBUF f16: balance ACT/DVE
                    on_act = mi < 2 or (mi == 2 and j % 2 == 0)
                    if on_act:
                        nc.scalar.copy(t_tiles[name][:, :], pT[:, :])
                    else:
                        nc.vector.tensor_copy(t_tiles[name][:, :], pT[:, :])

                # ---- stage 2: W-conv into grouped psum tiles ----
                ab_ps = psAB.tile([128, 1024], F32, tag="ab")
                cde_ps = psCDE.tile([128, 1536], F32, tag="cde")
                bw = _band_w(bands_t, j)
                n = won
                nc.tensor.matmul(ab_ps[0:n, 0:512], bw, t_tiles["si"][:, :],
                                 start=True, stop=True)
                nc.tensor.matmul(ab_ps[0:n, 512:1024], bw,
                                 t_tiles["sj"][:, :], start=True, stop=True)
                nc.tensor.matmul(cde_ps[0:n, 0:512], bw, t_tiles["sij"][:, :],
                                 start=True, stop=True)
                nc.tensor.matmul(cde_ps[0:n, 512:1024], bw,
                                 t_tiles["sii"][:, :], start=True, stop=True)
                nc.tensor.matmul(cde_ps[0:n, 1024:1536], bw,
                                 t_tiles["sjj"][:, :], start=True, stop=True)

                # ---- iteration filler + combine, ordered per-engine ----
                do_products = (hh == 0 and j >= 4)
                do_s = (not do_products) or j >= 5
                # DVE filler ahead of the ab_sb-dependent combine heads
                if do_s:
                    emit_s_lags()
                # combine heads
                ab_sb = ctmp.tile([128, 1024], BF16, tag="ab_sb")
                nc.scalar.copy(ab_sb[0:n, :], ab_ps[0:n, :])
                a_sb = ab_sb[0:n, 0:512]
                b_sb = ab_sb[0:n, 512:1024]
                pab = ctmp.tile([128, 1536], BF16, tag="pab")
                nc.vector.tensor_tensor(out=pab[0:n, 0:512], in0=a_sb,
                                        in1=b_sb, op=ALU.mult)
                # (A2|B2) in one wide square (src==src runs 1x; one op
                # beats two separate 1x squares)
                nc.vector.tensor_tensor(out=pab[0:n, 512:1536],
                                        in0=ab_sb[0:n, :],
                                        in1=ab_sb[0:n, :], op=ALU.mult)
                cij = ctmp.tile([128, 1536], BF16, tag="cij")
                nc.vector.scalar_tensor_tensor(
                    out=cij[0:n, :], in0=cde_ps[0:n, :], scalar=1.0,
                    in1=pab[0:n, :], op0=ALU.mult, op1=ALU.subtract)
                # ACT/GPS filler while GPS dd waits on the STT
                if do_products:
                    emit_products(j + 1)
                if do_s:
                    emit_s_sq()
                # combine tail
                dd = ctmp.tile([128, 512], BF16, tag="dd")
                nc.gpsimd.tensor_tensor(out=dd[0:n, :],
                                        in0=cij[0:n, 512:1024],
                                        in1=cij[0:n, 1024:1536], op=ALU.mult)
                rs = ctmp.tile([128, 512], BF16, tag="rs")
                _act_raw(nc, rs[0:n, :], dd[0:n, :], ACTF.Rsqrt)
                qj = ctmp.tile([128, 512], BF16, tag="qj")
                nc.gpsimd.tensor_tensor(out=qj[0:n, :], in0=cij[0:n, 0:512],
                                        in1=rs[0:n, :], op=ALU.mult)
                o4 = ctmp.tile([128, 512], F16, tag="o4")
                col = ACC_CC + 2 * j + hh
                nc.scalar.activation(o4[0:n, :], qj[0:n, :], ACTF.Square,
                                     accum_out=acc[0:n, col:col + 1])

        while s_state[1] < 16:
            emit_s_lags()
        while s_state[2] < 16:
            emit_s_sq()

        # ---------------- final partition reduction ------------------------
        pF = psT.tile([2, NACC], F32, tag="psT")
        nc.tensor.matmul(pF[:], onesp_t[:], acc[:], start=True, stop=True)
        outt = accp.tile([2, NACC], F32, tag="outt")
        nc.scalar.copy(outt[:], pF[:])
        nc.scalar.dma_start(part_d, outt[:])

    return


def _get_nc():
    if "nc" not in _nc_cache:
        nc = bass.Bass("TRN2", target_bir_lowering=False, debug=False)
        _build(nc)
        _legalize_waits(nc)
        _nc_cache["nc"] = nc
    return _nc_cache["nc"]


def _make_in_maps(I, J, s, consts):
    in_maps = []
    for b in range(I.shape[0]):
        m = {
            "I": np.ascontiguousarray(I[b, 0]),
            "J": np.ascontiguousarray(J[b, 0]),
            "s0": np.ascontiguousarray(s[b, 0]),
            "s1": np.ascontiguousarray(s[b, 1]),
            "sT0": np.ascontiguousarray(s[b, 0].T),
            "sT1": np.ascontiguousarray(s[b, 1].T),
        }
        m.update(_const_map(consts))
        in_maps.append(m)
    return in_maps


def kernel(I, J, s, sum_filt):
    B = I.shape[0]
    assert I.shape == (B, 1, H, W) and s.shape == (B, 2, H, W)
    nc = _get_nc()
    consts = _make_host_consts()

    in_maps = _make_in_maps(I, J, s, consts)
    res = bass_utils.run_bass_kernel_spmd(nc, in_maps,
                                          core_ids=list(range(B)))
    parts = np.stack([res.results[b]["partials"] for b in range(B)])
    parts = parts.astype(np.float64)  # [B, 2, NACC]

    s64 = s.astype(np.float64)
    cc_sum = float(parts[:, 0, ACC_CC:ACC_CC + 18].sum())
    lag_acc = parts[:, 0, ACC_LW:ACC_LW + 16].sum(axis=1)
    s2 = parts[:, 0, ACC_S2:ACC_S2 + 16].sum(axis=1)

    # seam term folded into the fused lag pass: s[.,1023]*sT-row heads
    seam = (s64[:, :, :, -1].reshape(B, 2, 8, 128) *
            s64[:, :, 0, :].reshape(B, 2, 8, 128)).sum(axis=(1, 2, 3))
    lag_sum = lag_acc - seam

    # edge corrections per core (both channels folded together)
    e_w = (s64[:, :, :, 0] ** 2).sum(axis=(1, 2)) + \
          (s64[:, :, :, -1] ** 2).sum(axis=(1, 2))
    e_h = (s64[:, :, 0, :] ** 2).sum(axis=(1, 2)) + \
          (s64[:, :, -1, :] ** 2).sum(axis=(1, 2))

    num = (4.0 * s2 - e_w - e_h - 2.0 * lag_sum).sum()
    cnt = B * 2 * H * (W - 1)

    ncc_loss = -cc_sum / (B * H * W)
    smooth = 0.5 * num / cnt * ALPHA
    total = ncc_loss + smooth
    return np.array([total, ncc_loss, smooth], dtype=np.float32)


# revision 27
# speedup vs baseline: 1.1308x; 1.0786x over previous
"""Trainium2 Bass kernel for LocalCrossCorrelationWithSmoothnessLoss.

Full inputs in, full output out. Pure data-parallel over batch (B=8 -> 8
NeuronCores); each core computes partial sums for its image; the host
combines them into the three scalar losses.

v2 design (vs baseline): cut elementwise passes + keep PE dense.
  products  sii/sjj via ACT Square (f32->f16), sij via DVE TT.  si/sj maps
            are NOT materialized: pass-1 uses the raw f32r I/J chunks as
            stationary with an f32r unscaled band moving.
  pass 1    fused H-conv + transpose on the PE (per map, 5 chunk-MMs
            accumulate into a 1-bank psum tile) -> T-copy to f16 SBUF
            (ACT/DVE split).
  stage 2   W-conv: band f16 stationary, T f16 moving; a,b land in one
            2-bank psum tile [*,1024]; c,d,e land in one 3-bank psum tile
            [*,1536] (adjacent banks).
  combine   ab_sb = ACT wide copy; P on GPS, (A2|B2) one wide DVE square;
            ONE wide DVE STT (cde_psum - PAB) -> (crossN|IvarN|JvarN)
            bf16; D = GPS TT, R = ACT Rsqrt, q = DVE TT,
            acc += q^2 (ACT Square accum).
  smooth    s^2 (ACT Square accum), lag_w (DVE STT accum), lag_h = free-
            axis DVE STT accum over the host-transposed sT input.

Output per core: [2, 82] partial sums (row 1 = partition-0-masked).
Host assembles the losses in float64.
"""
import sys
import numpy as np

sys.path.insert(0, "/opt/trn_rl_repo")

import ml_dtypes
import bass_rust
import concourse.bass as bass
import concourse.tile as tile
from concourse import mybir
from concourse import bass_utils
from concourse import tile_utils

F32 = mybir.dt.float32
F32R = mybir.dt.float32r
F16 = mybir.dt.float16
BF16 = mybir.dt.bfloat16
ALU = mybir.AluOpType
ACTF = mybir.ActivationFunctionType

H = 1024
W = 1024
PAD = 4
WIN = 81.0
ALPHA = 0.01

# r'-chunks for pass-1 (out range, in range). 512-aligned slices:
# {120,120,120,120,32} x 2.  in = out +- PAD clamped to [0, H].
RCH = []
for _lo in (0, 120, 240, 360, 480, 512, 632, 752, 872, 992):
    _n = 32 if _lo in (480, 992) else 120
    _ilo = max(0, _lo - PAD)
    _ihi = min(H, _lo + _n + PAD)
    RCH.append((_lo, _n, _ilo, _ihi - _ilo))
NRC = len(RCH)

# w'-chunks for stage-2: out w' range + the 128-wide stationary col window.
WCH = []
for _j in range(9):
    _olo = 120 * _j
    _on = min(120, W - _olo)
    _clo = 0 if _j == 0 else (W - 128 if _olo + _on + PAD > W else _olo - PAD)
    WCH.append((_olo, _on, _clo))
NWC = len(WCH)

# map order matters: stage-2 psum layout is a,b -> ab tile; c,d,e (sij,
# sii, sjj) -> cde tile thirds, matching the wide-STT operand layout.
MAPS = ("si", "sj", "sij", "sii", "sjj")

# accumulator columns
ACC_CC = 0          # 18: (j, half)
ACC_S2 = 18         # 16: (ch, tile)
ACC_LW = 34         # 16
ACC_SH = 50         # 32: (ch, tile, half)
NACC = 82

tile_utils.max_sbuf_usage = 207 * 1024

_nc_cache = {}


def _legalize_waits(nc, max_waits=1):
    """walrus accepts only one sync-wait per instruction; split extras
    onto same-engine NoOps placed just before."""
    ctr = 0
    for f in nc.m.functions:
        for bb in f.blocks:
            insts = bb.instructions
            i = 0
            while i < len(insts):
                ins = insts[i]
                si = ins.sync_info
                if si is None:
                    i += 1
                    continue
                w = list(si.on_wait)
                if len(w) <= max_waits:
                    i += 1
                    continue
                extra, keep = w[:-max_waits], w[-max_waits:]
                nops = []
                for j in range(0, len(extra), max_waits):
                    chunk = extra[j:j + max_waits]
                    nop = mybir.InstNoOp(name=f"I-wsplit-{ctr}", ins=[], outs=[])
                    ctr += 1
                    nop.engine = ins.engine
                    nop.sync_info = bass_rust.SyncInfo(on_wait=chunk, on_update=[])
                    nops.append(nop)
                ins.sync_info = bass_rust.SyncInfo(on_wait=keep,
                                                  on_update=list(si.on_update))
                insts[i:i] = nops
                i += len(nops) + 1


def _act_raw(nc, out, in_, func, scale=1.0, accum_out=None):
    """InstActivation without the bass Rsqrt/Reciprocal guard."""
    se = nc.scalar
    bias = nc.const_aps.scalar_like(0.0, in_)
    ins = [se.lower_ap(in_), se.lower_ap(bias),
           mybir.ImmediateValue(dtype=mybir.dt.float32, value=scale),
           mybir.ImmediateValue(dtype=mybir.dt.float32, value=0.0)]
    outs = [se.lower_ap(out)]
    if accum_out is not None:
        outs.append(se.lower_ap(accum_out))
    return se.add_instruction(mybir.InstActivation(
        name=nc.get_next_instruction_name(), func=func, ins=ins, outs=outs))


def _band(klo, kn, olo, on, scale):
    k = np.arange(klo, klo + kn)[:, None]
    m = np.arange(olo, olo + on)[None, :]
    return (np.abs(k - m) <= PAD).astype(np.float32) * scale


def _make_host_consts():
    # bands_f16 tile [128, 544]:
    #   cols   0:120  B0    = |k - m|     <= 4   (unscaled)
    #   cols 120:240  Bmid  = |k - 4 - m| <= 4   (unscaled)
    #   cols 240:304  B8    = |k - 64 - m| <= 4  (unscaled, stage-2 j=8)
    #   cols 304:424  B0s   = B0 * 81
    #   cols 424:544  Bmids = Bmid * 81
    bands = np.zeros((128, 544), dtype=np.float32)
    bands[:, 0:120] = _band(0, 128, 0, 120, 1.0)
    bands[:, 120:240] = _band(0, 128, 4, 120, 1.0)
    bands[:, 240:304] = _band(0, 128, 64, 64, 1.0)
    bands[:, 304:424] = _band(0, 128, 0, 120, WIN)
    bands[:, 424:544] = _band(0, 128, 4, 120, WIN)
    bands_f16 = bands.astype(np.float16)
    bands_bf = bands[:, 0:240].astype(ml_dtypes.bfloat16)

    # ones [128, 2]: col 0 full, col 1 masks partition 0
    onesp = np.ones((128, 2), dtype=np.float32)
    onesp[0, 1] = 0.0
    return bands_f16, bands_bf, onesp


def _const_map(consts):
    bands_f16, bands_bf, onesp = consts
    return {"bands": bands_f16, "bandsbf": bands_bf, "onesp": onesp}


def _band_r16(bands_t, c, scaled):
    """Moving f16 band AP for pass-1 r-chunk c: [r_in rows, out cols]."""
    olo, on, ilo, inn = RCH[c]
    if c == 0:
        base = 304 if scaled else 0
    else:
        base = 424 if scaled else 120
    return bands_t[0:inn, base:base + on]


def _band_rbf(bandsbf_t, c):
    olo, on, ilo, inn = RCH[c]
    base = 0 if c == 0 else 120
    return bandsbf_t[0:inn, base:base + on]


def _band_w(bands_t, j):
    """Stationary band AP for stage-2 w-chunk j: [128, out cols]."""
    olo, on, clo = WCH[j]
    if j == 0:
        return bands_t[0:128, 0:on]
    if olo - PAD == clo:
        return bands_t[0:128, 120:120 + on]
    return bands_t[0:128, 240:240 + on]


def _build(nc):
    I_d = nc.dram_tensor("I", [H, W], F32R, kind="ExternalInput").ap()
    J_d = nc.dram_tensor("J", [H, W], F32R, kind="ExternalInput").ap()
    s0_d = nc.dram_tensor("s0", [H, W], F32R, kind="ExternalInput").ap()
    s1_d = nc.dram_tensor("s1", [H, W], F32R, kind="ExternalInput").ap()
    sT0_d = nc.dram_tensor("sT0", [W, H], F32R, kind="ExternalInput").ap()
    sT1_d = nc.dram_tensor("sT1", [W, H], F32R, kind="ExternalInput").ap()
    bands_d = nc.dram_tensor("bands", [128, 544], F16,
                             kind="ExternalInput").ap()
    bandsbf_d = nc.dram_tensor("bandsbf", [128, 240], BF16,
                               kind="ExternalInput").ap()
    onesp_d = nc.dram_tensor("onesp", [128, 2], F32,
                             kind="ExternalInput").ap()
    part_d = nc.dram_tensor("partials", [2, NACC], F32,
                            kind="ExternalOutput").ap()

    from contextlib import ExitStack
    with tile.TileContext(nc) as tc, ExitStack() as ctx:
        consts = ctx.enter_context(tc.tile_pool(name="consts", bufs=1))
        inp = ctx.enter_context(tc.tile_pool(name="inp", bufs=1))
        xmap = ctx.enter_context(tc.tile_pool(name="xmap", bufs=1))
        tmap = ctx.enter_context(tc.tile_pool(name="tmap", bufs=2))
        ctmp = ctx.enter_context(tc.tile_pool(name="ctmp", bufs=2))
        spool = ctx.enter_context(tc.tile_pool(name="spool", bufs=3))
        sjunk = ctx.enter_context(tc.tile_pool(name="sjunk", bufs=3))
        accp = ctx.enter_context(tc.tile_pool(name="accp", bufs=1))
        psT = ctx.enter_context(tc.tile_pool(name="psT", bufs=3, space="PSUM"))
        psAB = ctx.enter_context(tc.tile_pool(name="psAB", bufs=1,
                                              space="PSUM"))
        psCDE = ctx.enter_context(tc.tile_pool(name="psCDE", bufs=1,
                                               space="PSUM"))

        bands_t = consts.tile([128, 544], F16)
        bandsbf_t = consts.tile([128, 240], BF16)
        onesp_t = consts.tile([128, 2], F32)
        nc.sync.dma_start(bands_t[:], bands_d)
        nc.sync.dma_start(bandsbf_t[:], bandsbf_d)
        nc.sync.dma_start(onesp_t[:], onesp_d)

        acc = accp.tile([128, NACC], F32)
        nc.vector.memset(acc[:], 0.0)

        # PE warm-up: ~4us of dummy matmuls while input DMA is in flight,
        # so HAM un-throttles (K=8/8) before the real work arrives.
        warm_ps = psT.tile([128, 512], F32, tag="psT")
        for wk in range(9):
            nc.tensor.matmul(warm_ps[0:120, 0:512], bands_t[0:128, 0:120],
                             bands_t[0:128, 0:512], start=(wk == 0),
                             stop=(wk == 8), skip_group_check=True)

        # ---------------- emission helpers --------------------------------
        def load_rows(dst, src, r0, n, eng=None):
            eng = eng or nc.sync
            eng.dma_start(dst[0:n, :], src[r0:r0 + n, :])

        s_issued = []
        s_state = [0, 0, 0]  # issued, lags done, squares done

        def issue_s():
            k = s_state[0]
            if k >= 16:
                return
            s_state[0] += 1
            ch, t = k // 8, k % 8
            s_d = s0_d if ch == 0 else s1_d
            sT_d = sT0_d if ch == 0 else sT1_d
            comb = spool.tile([128, W + H], F32R, tag="s_in")
            nc.sync.dma_start(comb[0:128, 0:W], s_d[128 * t:128 * t + 128, :])
            nc.scalar.dma_start(comb[0:128, W:W + H],
                                sT_d[128 * t:128 * t + 128, :])
            s_issued.append(comb)

        def emit_s_lags():
            """one fused lag pass: (lag_w + lag_h + seam) STT accum over the
            combined s|sT tile; the host subtracts the seam term."""
            k = s_state[1]
            if k >= 16:
                return
            while s_state[0] < min(16, k + 3):
                issue_s()
            s_state[1] += 1
            comb = s_issued[k]
            o2 = sjunk.tile([128, W + H], F16, tag="junk")
            nc.vector.scalar_tensor_tensor(
                out=o2[:, 0:W + H - 1], in0=comb[:, 1:W + H], scalar=1.0,
                in1=comb[:, 0:W + H - 1], op0=ALU.mult, op1=ALU.mult,
                accum_out=acc[:, ACC_LW + k:ACC_LW + k + 1])

        def emit_s_sq():
            """ACT half: s^2 Square-accum, filler between ab_sb and rs."""
            k = s_state[2]
            if k >= 16 or k >= s_state[1]:
                return
            s_state[2] += 1
            comb = s_issued[k]
            o1 = sjunk.tile([128, W + H], F16, tag="junk")
            nc.scalar.activation(o1[:, 0:W], comb[:, 0:W], ACTF.Square,
                                 accum_out=acc[:, ACC_S2 + k:ACC_S2 + k + 1])

        # ---------------- products (emitted per r-chunk) -------------------
        x_tiles = {}       # (name, c) -> f16 product map tiles
        raw_tiles = {}     # c -> (I_t, J_t) f32r, live for the whole half
        chunk_loaded = set()

        def issue_chunk(c):
            if c in chunk_loaded:
                return
            chunk_loaded.add(c)
            olo, on, ilo, inn = RCH[c]
            I_t = inp.tile([128, W], F32R, tag=f"I_in_{c}",
                           name=f"I_in_{c}")
            J_t = inp.tile([128, W], F32R, tag=f"J_in_{c}",
                           name=f"J_in_{c}")
            load_rows(I_t, I_d, ilo, inn)
            load_rows(J_t, J_d, ilo, inn, eng=nc.scalar)
            raw_tiles[c] = (I_t, J_t)

        def emit_products(c):
            olo, on, ilo, inn = RCH[c]
            issue_chunk(c)
            I_t, J_t = raw_tiles[c]
            for name in ("sij", "sii", "sjj"):
                xtag = f"X_{name}_c4c9" if c in (4, 9) else f"X_{name}_{c}"
                x_tiles[(name, c)] = xmap.tile([128, W], F16, tag=xtag,
                                               name=f"X_{name}_{c}")
            nc.scalar.square(x_tiles[("sii", c)][0:inn, :], I_t[0:inn, :])
            nc.scalar.square(x_tiles[("sjj", c)][0:inn, :], J_t[0:inn, :])
            nc.gpsimd.tensor_tensor(out=x_tiles[("sij", c)][0:inn, :],
                                     in0=I_t[0:inn, :], in1=J_t[0:inn, :],
                                     op=ALU.mult)

        for c in range(5):
            issue_chunk(c)
        for c in range(5):
            emit_products(c)

        # ------------- main loop: r-half outer, w-chunks inner -------------
        # combine tail (dd/rs/qj/accSq) is deferred one iteration so the
        # next iteration's T-copies/stage-2 never queue behind it.
        pending_tail = []

        def emit_tail():
            if not pending_tail:
                return
            cij, n, col = pending_tail.pop()
            dd = ctmp.tile([128, 512], BF16, tag="dd")
            nc.gpsimd.tensor_tensor(out=dd[0:n, :],
                                    in0=cij[0:n, 512:1024],
                                    in1=cij[0:n, 1024:1536], op=ALU.mult)
            rs = ctmp.tile([128, 512], BF16, tag="rs")
            _act_raw(nc, rs[0:n, :], dd[0:n, :], ACTF.Rsqrt)
            qj = ctmp.tile([128, 512], BF16, tag="qj")
            nc.gpsimd.tensor_tensor(out=qj[0:n, :], in0=cij[0:n, 0:512],
                                    in1=rs[0:n, :], op=ALU.mult)
            o4 = ctmp.tile([128, 512], F16, tag="o4")
            nc.scalar.activation(o4[0:n, :], qj[0:n, :], ACTF.Square,
                                 accum_out=acc[0:n, col:col + 1])

        for hh in range(2):
            cs = list(range(5 * hh, 5 * hh + 5))
            for j, (wolo, won, wclo) in enumerate(WCH):
                wsl = slice(wclo, wclo + 128)
                t_tiles = {}
                # ---- pass 1 + T-copy, per map ----
                for mi, name in enumerate(MAPS):
                    t_tiles[name] = tmap.tile([128, 512], F16,
                                              tag=f"T_{name}",
                                              name=f"T_{name}_{j}_{hh}")
                    pT = psT.tile([128, 512], F32, tag="psT")
                    bsl = slice(2 * wclo + 1, 2 * (wclo + 128), 2)
                    for c in cs:
                        olo, on, ilo, inn = RCH[c]
                        if name == "si":
                            stat = raw_tiles[c][0].bitcast(BF16)[0:inn, bsl]
                            mov = _band_rbf(bandsbf_t, c)
                        elif name == "sj":
                            stat = raw_tiles[c][1].bitcast(BF16)[0:inn, bsl]
                            mov = _band_rbf(bandsbf_t, c)
                        else:
                            stat = x_tiles[(name, c)][0:inn, wsl]
                            mov = _band_r16(bands_t, c, True)
                        nc.tensor.matmul(
                            pT[0:128, olo - 512 * hh:olo - 512 * hh + on],
                            stat, mov,
                            start=(c == cs[0]), stop=(c == cs[-1]),
                            skip_group_check=True)
                    # T-copy psum -> SBUF f16: balance ACT/DVE
                    on_act = mi < 2 or (mi == 2 and j % 2 == 0)
                    if on_act:
                        nc.scalar.copy(t_tiles[name][:, :], pT[:, :])
                    else:
                        nc.vector.tensor_copy(t_tiles[name][:, :], pT[:, :])

                # ---- stage 2: W-conv into grouped psum tiles ----
                ab_ps = psAB.tile([128, 1024], F32, tag="ab")
                cde_ps = psCDE.tile([128, 1536], F32, tag="cde")
                bw = _band_w(bands_t, j)
                n = won
                nc.tensor.matmul(ab_ps[0:n, 0:512], bw, t_tiles["si"][:, :],
                                 start=True, stop=True)
                nc.tensor.matmul(ab_ps[0:n, 512:1024], bw,
                                 t_tiles["sj"][:, :], start=True, stop=True)
                nc.tensor.matmul(cde_ps[0:n, 0:512], bw, t_tiles["sij"][:, :],
                                 start=True, stop=True)
                nc.tensor.matmul(cde_ps[0:n, 512:1024], bw,
                                 t_tiles["sii"][:, :], start=True, stop=True)
                nc.tensor.matmul(cde_ps[0:n, 1024:1536], bw,
                                 t_tiles["sjj"][:, :], start=True, stop=True)

                # ---- combine heads (release psAB/psCDE fast) ----
                ab_sb = ctmp.tile([128, 1024], BF16, tag="ab_sb")
                nc.scalar.copy(ab_sb[0:n, :], ab_ps[0:n, :])
                pab = ctmp.tile([128, 1536], BF16, tag="pab")
                nc.vector.tensor_tensor(out=pab[0:n, 0:512],
                                        in0=ab_sb[0:n, 0:512],
                                        in1=ab_sb[0:n, 512:1024],
                                        op=ALU.mult)
                nc.vector.tensor_tensor(out=pab[0:n, 512:1536],
                                        in0=ab_sb[0:n, :],
                                        in1=ab_sb[0:n, :], op=ALU.mult)
                cij = ctmp.tile([128, 1536], BF16, tag="cij")
                nc.vector.scalar_tensor_tensor(
                    out=cij[0:n, :], in0=cde_ps[0:n, :], scalar=1.0,
                    in1=pab[0:n, :], op0=ALU.mult, op1=ALU.subtract)

                # ---- previous iteration's tail + fillers ----
                emit_tail()
                do_products = (hh == 0 and j >= 4)
                do_s = (not do_products) or j >= 5
                if do_products:
                    emit_products(j + 1)
                if do_s:
                    emit_s_lags()
                    emit_s_sq()
                pending_tail.append((cij, n, ACC_CC + 2 * j + hh))

        emit_tail()
        while s_state[1] < 16:
            emit_s_lags()
        while s_state[2] < 16:
            emit_s_sq()

        # ---------------- final partition reduction ------------------------
        pF = psT.tile([2, NACC], F32, tag="psT")
        nc.tensor.matmul(pF[:], onesp_t[:], acc[:], start=True, stop=True)
        outt = accp.tile([2, NACC], F32, tag="outt")
        nc.scalar.copy(outt[:], pF[:])
        nc.scalar.dma_start(part_d, outt[:])

    return


def _get_nc():
    if "nc" not in _nc_cache:
        nc = bass.Bass("TRN2", target_bir_lowering=False, debug=False)
        _build(nc)
        _legalize_waits(nc)
        _nc_cache["nc"] = nc
    return _nc_cache["nc"]


def _make_in_maps(I, J, s, consts):
    in_maps = []
    for b in range(I.shape[0]):
        m = {
            "I": np.ascontiguousarray(I[b, 0]),
            "J": np.ascontiguousarray(J[b, 0]),
            "s0": np.ascontiguousarray(s[b, 0]),
            "s1": np.ascontiguousarray(s[b, 1]),
            "sT0": np.ascontiguousarray(s[b, 0].T),
            "sT1": np.ascontiguousarray(s[b, 1].T),
        }
        m.update(_const_map(consts))
        in_maps.append(m)
    return in_maps


def kernel(I, J, s, sum_filt):
    B = I.shape[0]
    assert I.shape == (B, 1, H, W) and s.shape == (B, 2, H, W)
    nc = _get_nc()
    consts = _make_host_consts()

    in_maps = _make_in_maps(I, J, s, consts)
    res = bass_utils.run_bass_kernel_spmd(nc, in_maps,
                                          core_ids=list(range(B)))
    parts = np.stack([res.results[b]["partials"] for b in range(B)])
    parts = parts.astype(np.float64)  # [B, 2, NACC]

    s64 = s.astype(np.float64)
    cc_sum = float(parts[:, 0, ACC_CC:ACC_CC + 18].sum())
    lag_acc = parts[:, 0, ACC_LW:ACC_LW + 16].sum(axis=1)
    s2 = parts[:, 0, ACC_S2:ACC_S2 + 16].sum(axis=1)

    # seam term folded into the fused lag pass: s[.,1023]*sT-row heads
    seam = (s64[:, :, :, -1].reshape(B, 2, 8, 128) *
            s64[:, :, 0, :].reshape(B, 2, 8, 128)).sum(axis=(1, 2, 3))
    lag_sum = lag_acc - seam

    # edge corrections per core (both channels folded together)
    e_w = (s64[:, :, :, 0] ** 2).sum(axis=(1, 2)) + \
          (s64[:, :, :, -1] ** 2).sum(axis=(1, 2))
    e_h = (s64[:, :, 0, :] ** 2).sum(axis=(1, 2)) + \
          (s64[:, :, -1, :] ** 2).sum(axis=(1, 2))

    num = (4.0 * s2 - e_w - e_h - 2.0 * lag_sum).sum()
    cnt = B * 2 * H * (W - 1)

    ncc_loss = -cc_sum / (B * H * W)
    smooth = 0.5 * num / cnt * ALPHA
    total = ncc_loss + smooth
    return np.array([total, ncc_loss, smooth], dtype=np.float32)


# revision 28
# speedup vs baseline: 1.1412x; 1.0091x over previous
"""Trainium2 Bass kernel for LocalCrossCorrelationWithSmoothnessLoss.

Full inputs in, full output out. Pure data-parallel over batch (B=8 -> 8
NeuronCores); each core computes partial sums for its image; the host
combines them into the three scalar losses.

v2 design (vs baseline): cut elementwise passes + keep PE dense.
  products  sii/sjj via ACT Square (f32->f16), sij via DVE TT.  si/sj maps
            are NOT materialized: pass-1 uses the raw f32r I/J chunks as
            stationary with an f32r unscaled band moving.
  pass 1    fused H-conv + transpose on the PE (per map, 5 chunk-MMs
            accumulate into a 1-bank psum tile) -> T-copy to f16 SBUF
            (ACT/DVE split).
  stage 2   W-conv: band f16 stationary, T f16 moving; a,b land in one
            2-bank psum tile [*,1024]; c,d,e land in one 3-bank psum tile
            [*,1536] (adjacent banks).
  combine   ab_sb = ACT wide copy; P on GPS, (A2|B2) one wide DVE square;
            ONE wide DVE STT (cde_psum - PAB) -> (crossN|IvarN|JvarN)
            bf16; D = GPS TT, R = ACT Rsqrt, q = DVE TT,
            acc += q^2 (ACT Square accum).
  smooth    s^2 (ACT Square accum), lag_w (DVE STT accum), lag_h = free-
            axis DVE STT accum over the host-transposed sT input.

Output per core: [2, 82] partial sums (row 1 = partition-0-masked).
Host assembles the losses in float64.
"""
import sys
import numpy as np

sys.path.insert(0, "/opt/trn_rl_repo")

import ml_dtypes
import bass_rust
import concourse.bass as bass
import concourse.tile as tile
from concourse import mybir
from concourse import bass_utils
from concourse import tile_utils

F32 = mybir.dt.float32
F32R = mybir.dt.float32r
F16 = mybir.dt.float16
BF16 = mybir.dt.bfloat16
ALU = mybir.AluOpType
ACTF = mybir.ActivationFunctionType

H = 1024
W = 1024
PAD = 4
WIN = 81.0
ALPHA = 0.01

# r'-chunks for pass-1 (out range, in range). 512-aligned slices:
# {120,120,120,120,32} x 2.  in = out +- PAD clamped to [0, H].
RCH = []
for _lo in (0, 120, 240, 360, 480, 512, 632, 752, 872, 992):
    _n = 32 if _lo in (480, 992) else 120
    _ilo = max(0, _lo - PAD)
    _ihi = min(H, _lo + _n + PAD)
    RCH.append((_lo, _n, _ilo, _ihi - _ilo))
NRC = len(RCH)

# w'-chunks for stage-2: out w' range + the 128-wide stationary col window.
WCH = []
for _j in range(9):
    _olo = 120 * _j
    _on = min(120, W - _olo)
    _clo = 0 if _j == 0 else (W - 128 if _olo + _on + PAD > W else _olo - PAD)
    WCH.append((_olo, _on, _clo))
NWC = len(WCH)

# map order matters: stage-2 psum layout is a,b -> ab tile; c,d,e (sij,
# sii, sjj) -> cde tile thirds, matching the wide-STT operand layout.
MAPS = ("si", "sj", "sij", "sii", "sjj")

# accumulator columns
ACC_CC = 0          # 18: (j, half)
ACC_S2 = 18         # 16: (ch, tile)
ACC_LW = 34         # 16
ACC_SH = 50         # 32: (ch, tile, half)
NACC = 82

tile_utils.max_sbuf_usage = 207 * 1024

_nc_cache = {}


def _legalize_waits(nc, max_waits=1):
    """walrus accepts only one sync-wait per instruction; split extras
    onto same-engine NoOps placed just before."""
    ctr = 0
    for f in nc.m.functions:
        for bb in f.blocks:
            insts = bb.instructions
            i = 0
            while i < len(insts):
                ins = insts[i]
                si = ins.sync_info
                if si is None:
                    i += 1
                    continue
                w = list(si.on_wait)
                if len(w) <= max_waits:
                    i += 1
                    continue
                extra, keep = w[:-max_waits], w[-max_waits:]
                nops = []
                for j in range(0, len(extra), max_waits):
                    chunk = extra[j:j + max_waits]
                    nop = mybir.InstNoOp(name=f"I-wsplit-{ctr}", ins=[], outs=[])
                    ctr += 1
                    nop.engine = ins.engine
                    nop.sync_info = bass_rust.SyncInfo(on_wait=chunk, on_update=[])
                    nops.append(nop)
                ins.sync_info = bass_rust.SyncInfo(on_wait=keep,
                                                  on_update=list(si.on_update))
                insts[i:i] = nops
                i += len(nops) + 1


def _act_raw(nc, out, in_, func, scale=1.0, accum_out=None):
    """InstActivation without the bass Rsqrt/Reciprocal guard."""
    se = nc.scalar
    bias = nc.const_aps.scalar_like(0.0, in_)
    ins = [se.lower_ap(in_), se.lower_ap(bias),
           mybir.ImmediateValue(dtype=mybir.dt.float32, value=scale),
           mybir.ImmediateValue(dtype=mybir.dt.float32, value=0.0)]
    outs = [se.lower_ap(out)]
    if accum_out is not None:
        outs.append(se.lower_ap(accum_out))
    return se.add_instruction(mybir.InstActivation(
        name=nc.get_next_instruction_name(), func=func, ins=ins, outs=outs))


def _band(klo, kn, olo, on, scale):
    k = np.arange(klo, klo + kn)[:, None]
    m = np.arange(olo, olo + on)[None, :]
    return (np.abs(k - m) <= PAD).astype(np.float32) * scale


def _make_host_consts():
    # bands_f16 tile [128, 544]:
    #   cols   0:120  B0    = |k - m|     <= 4   (unscaled)
    #   cols 120:240  Bmid  = |k - 4 - m| <= 4   (unscaled)
    #   cols 240:304  B8    = |k - 64 - m| <= 4  (unscaled, stage-2 j=8)
    #   cols 304:424  B0s   = B0 * 81
    #   cols 424:544  Bmids = Bmid * 81
    bands = np.zeros((128, 544), dtype=np.float32)
    bands[:, 0:120] = _band(0, 128, 0, 120, 1.0)
    bands[:, 120:240] = _band(0, 128, 4, 120, 1.0)
    bands[:, 240:304] = _band(0, 128, 64, 64, 1.0)
    bands[:, 304:424] = _band(0, 128, 0, 120, WIN)
    bands[:, 424:544] = _band(0, 128, 4, 120, WIN)
    bands_f16 = bands.astype(np.float16)
    bands_bf = bands[:, 0:240].astype(ml_dtypes.bfloat16)

    # ones [128, 2]: col 0 full, col 1 masks partition 0
    onesp = np.ones((128, 2), dtype=np.float32)
    onesp[0, 1] = 0.0
    return bands_f16, bands_bf, onesp


def _const_map(consts):
    bands_f16, bands_bf, onesp = consts
    return {"bands": bands_f16, "bandsbf": bands_bf, "onesp": onesp}


def _band_r16(bands_t, c, scaled):
    """Moving f16 band AP for pass-1 r-chunk c: [r_in rows, out cols]."""
    olo, on, ilo, inn = RCH[c]
    if c == 0:
        base = 304 if scaled else 0
    else:
        base = 424 if scaled else 120
    return bands_t[0:inn, base:base + on]


def _band_rbf(bandsbf_t, c):
    olo, on, ilo, inn = RCH[c]
    base = 0 if c == 0 else 120
    return bandsbf_t[0:inn, base:base + on]


def _band_w(bands_t, j):
    """Stationary band AP for stage-2 w-chunk j: [128, out cols]."""
    olo, on, clo = WCH[j]
    if j == 0:
        return bands_t[0:128, 0:on]
    if olo - PAD == clo:
        return bands_t[0:128, 120:120 + on]
    return bands_t[0:128, 240:240 + on]


def _build(nc):
    I_d = nc.dram_tensor("I", [H, W], F32R, kind="ExternalInput").ap()
    J_d = nc.dram_tensor("J", [H, W], F32R, kind="ExternalInput").ap()
    s0_d = nc.dram_tensor("s0", [H, W], BF16, kind="ExternalInput").ap()
    s1_d = nc.dram_tensor("s1", [H, W], BF16, kind="ExternalInput").ap()
    sT0_d = nc.dram_tensor("sT0", [W, H], BF16, kind="ExternalInput").ap()
    sT1_d = nc.dram_tensor("sT1", [W, H], BF16, kind="ExternalInput").ap()
    bands_d = nc.dram_tensor("bands", [128, 544], F16,
                             kind="ExternalInput").ap()
    bandsbf_d = nc.dram_tensor("bandsbf", [128, 240], BF16,
                               kind="ExternalInput").ap()
    onesp_d = nc.dram_tensor("onesp", [128, 2], F32,
                             kind="ExternalInput").ap()
    part_d = nc.dram_tensor("partials", [2, NACC], F32,
                            kind="ExternalOutput").ap()

    from contextlib import ExitStack
    with tile.TileContext(nc) as tc, ExitStack() as ctx:
        consts = ctx.enter_context(tc.tile_pool(name="consts", bufs=1))
        inp = ctx.enter_context(tc.tile_pool(name="inp", bufs=1))
        xmap = ctx.enter_context(tc.tile_pool(name="xmap", bufs=1))
        tmap = ctx.enter_context(tc.tile_pool(name="tmap", bufs=2))
        ctmp = ctx.enter_context(tc.tile_pool(name="ctmp", bufs=2))
        spool = ctx.enter_context(tc.tile_pool(name="spool", bufs=6))
        sjunk = ctx.enter_context(tc.tile_pool(name="sjunk", bufs=3))
        accp = ctx.enter_context(tc.tile_pool(name="accp", bufs=1))
        psT = ctx.enter_context(tc.tile_pool(name="psT", bufs=3, space="PSUM"))
        psAB = ctx.enter_context(tc.tile_pool(name="psAB", bufs=1,
                                              space="PSUM"))
        psCDE = ctx.enter_context(tc.tile_pool(name="psCDE", bufs=1,
                                               space="PSUM"))

        bands_t = consts.tile([128, 544], F16)
        bandsbf_t = consts.tile([128, 240], BF16)
        onesp_t = consts.tile([128, 2], F32)
        nc.sync.dma_start(bands_t[:], bands_d)
        nc.sync.dma_start(bandsbf_t[:], bandsbf_d)
        nc.sync.dma_start(onesp_t[:], onesp_d)

        acc = accp.tile([128, NACC], F32)
        nc.vector.memset(acc[:], 0.0)

        # PE warm-up: ~4us of dummy matmuls while input DMA is in flight,
        # so HAM un-throttles (K=8/8) before the real work arrives.
        warm_ps = psT.tile([128, 512], F32, tag="psT")
        for wk in range(9):
            nc.tensor.matmul(warm_ps[0:120, 0:512], bands_t[0:128, 0:120],
                             bands_t[0:128, 0:512], start=(wk == 0),
                             stop=(wk == 8), skip_group_check=True)

        # ---------------- emission helpers --------------------------------
        def load_rows(dst, src, r0, n, eng=None):
            eng = eng or nc.sync
            eng.dma_start(dst[0:n, :], src[r0:r0 + n, :])

        s_issued = []
        s_state = [0, 0, 0]  # issued, lags done, squares done

        def issue_s():
            k = s_state[0]
            if k >= 16:
                return
            s_state[0] += 1
            ch, t = k // 8, k % 8
            s_d = s0_d if ch == 0 else s1_d
            sT_d = sT0_d if ch == 0 else sT1_d
            comb = spool.tile([128, W + H], BF16, tag="s_in")
            nc.sync.dma_start(comb[0:128, 0:W], s_d[128 * t:128 * t + 128, :])
            nc.scalar.dma_start(comb[0:128, W:W + H],
                                sT_d[128 * t:128 * t + 128, :])
            s_issued.append(comb)

        def emit_s_lags():
            """one fused lag pass: (lag_w + lag_h + seam) STT accum over the
            combined s|sT tile; the host subtracts the seam term."""
            k = s_state[1]
            if k >= 16:
                return
            while s_state[0] < min(16, k + 6):
                issue_s()
            s_state[1] += 1
            comb = s_issued[k]
            o2 = sjunk.tile([128, W + H], F16, tag="junk")
            nc.vector.scalar_tensor_tensor(
                out=o2[:, 0:W + H - 1], in0=comb[:, 1:W + H], scalar=1.0,
                in1=comb[:, 0:W + H - 1], op0=ALU.mult, op1=ALU.mult,
                accum_out=acc[:, ACC_LW + k:ACC_LW + k + 1])

        def emit_s_sq():
            """ACT half: s^2 Square-accum, filler between ab_sb and rs."""
            k = s_state[2]
            if k >= 16 or k >= s_state[1]:
                return
            s_state[2] += 1
            comb = s_issued[k]
            o1 = sjunk.tile([128, W + H], F16, tag="junk")
            nc.scalar.activation(o1[:, 0:W], comb[:, 0:W], ACTF.Square,
                                 accum_out=acc[:, ACC_S2 + k:ACC_S2 + k + 1])

        # ---------------- products (emitted per r-chunk) -------------------
        x_tiles = {}       # (name, c) -> f16 product map tiles
        raw_tiles = {}     # c -> (I_t, J_t) f32r, live for the whole half
        chunk_loaded = set()

        def issue_chunk(c):
            if c in chunk_loaded:
                return
            chunk_loaded.add(c)
            olo, on, ilo, inn = RCH[c]
            I_t = inp.tile([128, W], F32R, tag=f"I_in_{c}",
                           name=f"I_in_{c}")
            J_t = inp.tile([128, W], F32R, tag=f"J_in_{c}",
                           name=f"J_in_{c}")
            load_rows(I_t, I_d, ilo, inn)
            load_rows(J_t, J_d, ilo, inn, eng=nc.scalar)
            raw_tiles[c] = (I_t, J_t)

        def emit_products(c):
            olo, on, ilo, inn = RCH[c]
            issue_chunk(c)
            I_t, J_t = raw_tiles[c]
            for name in ("sij", "sii", "sjj"):
                xtag = f"X_{name}_c4c9" if c in (4, 9) else f"X_{name}_{c}"
                x_tiles[(name, c)] = xmap.tile([128, W], F16, tag=xtag,
                                               name=f"X_{name}_{c}")
            nc.scalar.square(x_tiles[("sii", c)][0:inn, :], I_t[0:inn, :])
            nc.scalar.square(x_tiles[("sjj", c)][0:inn, :], J_t[0:inn, :])
            nc.gpsimd.tensor_tensor(out=x_tiles[("sij", c)][0:inn, :],
                                     in0=I_t[0:inn, :], in1=J_t[0:inn, :],
                                     op=ALU.mult)

        for c in range(10):
            issue_chunk(c)
        for c in range(5):
            emit_products(c)

        # ------------- main loop: r-half outer, w-chunks inner -------------
        # combine tail (dd/rs/qj/accSq) is deferred one iteration so the
        # next iteration's T-copies/stage-2 never queue behind it.
        pending_tail = []

        def emit_tail():
            if not pending_tail:
                return
            cij, n, col = pending_tail.pop()
            dd = ctmp.tile([128, 512], BF16, tag="dd")
            nc.gpsimd.tensor_tensor(out=dd[0:n, :],
                                    in0=cij[0:n, 512:1024],
                                    in1=cij[0:n, 1024:1536], op=ALU.mult)
            rs = ctmp.tile([128, 512], BF16, tag="rs")
            _act_raw(nc, rs[0:n, :], dd[0:n, :], ACTF.Rsqrt)
            qj = ctmp.tile([128, 512], BF16, tag="qj")
            nc.gpsimd.tensor_tensor(out=qj[0:n, :], in0=cij[0:n, 0:512],
                                    in1=rs[0:n, :], op=ALU.mult)
            o4 = ctmp.tile([128, 512], F16, tag="o4")
            nc.scalar.activation(o4[0:n, :], qj[0:n, :], ACTF.Square,
                                 accum_out=acc[0:n, col:col + 1])

        for hh in range(2):
            cs = list(range(5 * hh, 5 * hh + 5))
            for j, (wolo, won, wclo) in enumerate(WCH):
                wsl = slice(wclo, wclo + 128)
                t_tiles = {}
                # ---- pass 1 + T-copy, per map ----
                for mi, name in enumerate(MAPS):
                    t_tiles[name] = tmap.tile([128, 512], F16,
                                              tag=f"T_{name}",
                                              name=f"T_{name}_{j}_{hh}")
                    pT = psT.tile([128, 512], F32, tag="psT")
                    bsl = slice(2 * wclo + 1, 2 * (wclo + 128), 2)
                    for c in cs:
                        olo, on, ilo, inn = RCH[c]
                        if name == "si":
                            stat = raw_tiles[c][0].bitcast(BF16)[0:inn, bsl]
                            mov = _band_rbf(bandsbf_t, c)
                        elif name == "sj":
                            stat = raw_tiles[c][1].bitcast(BF16)[0:inn, bsl]
                            mov = _band_rbf(bandsbf_t, c)
                        else:
                            stat = x_tiles[(name, c)][0:inn, wsl]
                            mov = _band_r16(bands_t, c, True)
                        nc.tensor.matmul(
                            pT[0:128, olo - 512 * hh:olo - 512 * hh + on],
                            stat, mov,
                            start=(c == cs[0]), stop=(c == cs[-1]),
                            skip_group_check=True)
                    # T-copy psum -> SBUF f16: balance ACT/DVE
                    on_act = mi < 2 or (mi == 2 and j % 2 == 0)
                    if on_act:
                        nc.scalar.copy(t_tiles[name][:, :], pT[:, :])
                    else:
                        nc.vector.tensor_copy(t_tiles[name][:, :], pT[:, :])

                # ---- stage 2: W-conv into grouped psum tiles ----
                ab_ps = psAB.tile([128, 1024], F32, tag="ab")
                cde_ps = psCDE.tile([128, 1536], F32, tag="cde")
                bw = _band_w(bands_t, j)
                n = won
                nc.tensor.matmul(ab_ps[0:n, 0:512], bw, t_tiles["si"][:, :],
                                 start=True, stop=True)
                nc.tensor.matmul(ab_ps[0:n, 512:1024], bw,
                                 t_tiles["sj"][:, :], start=True, stop=True)
                nc.tensor.matmul(cde_ps[0:n, 0:512], bw, t_tiles["sij"][:, :],
                                 start=True, stop=True)
                nc.tensor.matmul(cde_ps[0:n, 512:1024], bw,
                                 t_tiles["sii"][:, :], start=True, stop=True)
                nc.tensor.matmul(cde_ps[0:n, 1024:1536], bw,
                                 t_tiles["sjj"][:, :], start=True, stop=True)

                # ---- combine heads (release psAB/psCDE fast) ----
                ab_sb = ctmp.tile([128, 1024], BF16, tag="ab_sb")
                nc.scalar.copy(ab_sb[0:n, :], ab_ps[0:n, :])
                pab = ctmp.tile([128, 1536], BF16, tag="pab")
                nc.vector.tensor_tensor(out=pab[0:n, 0:512],
                                        in0=ab_sb[0:n, 0:512],
                                        in1=ab_sb[0:n, 512:1024],
                                        op=ALU.mult)
                nc.vector.tensor_tensor(out=pab[0:n, 512:1536],
                                        in0=ab_sb[0:n, :],
                                        in1=ab_sb[0:n, :], op=ALU.mult)
                cij = ctmp.tile([128, 1536], BF16, tag="cij")
                nc.vector.scalar_tensor_tensor(
                    out=cij[0:n, :], in0=cde_ps[0:n, :], scalar=1.0,
                    in1=pab[0:n, :], op0=ALU.mult, op1=ALU.subtract)

                # ---- previous iteration's tail + fillers ----
                emit_tail()
                do_products = (hh == 0 and j >= 4)
                do_s = (not do_products) or j >= 5
                if do_products:
                    emit_products(j + 1)
                if do_s:
                    emit_s_lags()
                    emit_s_sq()
                pending_tail.append((cij, n, ACC_CC + 2 * j + hh))

        emit_tail()
        while s_state[1] < 16:
            emit_s_lags()
        while s_state[2] < 16:
            emit_s_sq()

        # ---------------- final partition reduction ------------------------
        pF = psT.tile([2, NACC], F32, tag="psT")
        nc.tensor.matmul(pF[:], onesp_t[:], acc[:], start=True, stop=True)
        outt = accp.tile([2, NACC], F32, tag="outt")
        nc.scalar.copy(outt[:], pF[:])
        nc.scalar.dma_start(part_d, outt[:])

    return


def _get_nc():
    if "nc" not in _nc_cache:
        nc = bass.Bass("TRN2", target_bir_lowering=False, debug=False)
        _build(nc)
        _legalize_waits(nc)
        _nc_cache["nc"] = nc
    return _nc_cache["nc"]


def _make_in_maps(I, J, s, consts):
    in_maps = []
    for b in range(I.shape[0]):
        m = {
            "I": np.ascontiguousarray(I[b, 0]),
            "J": np.ascontiguousarray(J[b, 0]),
            "s0": s[b, 0].astype(ml_dtypes.bfloat16),
            "s1": s[b, 1].astype(ml_dtypes.bfloat16),
            "sT0": np.ascontiguousarray(s[b, 0].T).astype(ml_dtypes.bfloat16),
            "sT1": np.ascontiguousarray(s[b, 1].T).astype(ml_dtypes.bfloat16),
        }
        m.update(_const_map(consts))
        in_maps.append(m)
    return in_maps


def kernel(I, J, s, sum_filt):
    B = I.shape[0]
    assert I.shape == (B, 1, H, W) and s.shape == (B, 2, H, W)
    nc = _get_nc()
    consts = _make_host_consts()

    in_maps = _make_in_maps(I, J, s, consts)
    res = bass_utils.run_bass_kernel_spmd(nc, in_maps,
                                          core_ids=list(range(B)))
    parts = np.stack([res.results[b]["partials"] for b in range(B)])
    parts = parts.astype(np.float64)  # [B, 2, NACC]

    s64 = s.astype(np.float64)
    cc_sum = float(parts[:, 0, ACC_CC:ACC_CC + 18].sum())
    lag_acc = parts[:, 0, ACC_LW:ACC_LW + 16].sum(axis=1)
    s2 = parts[:, 0, ACC_S2:ACC_S2 + 16].sum(axis=1)

    # seam term folded into the fused lag pass: s[.,1023]*sT-row heads
    seam = (s64[:, :, :, -1].reshape(B, 2, 8, 128) *
            s64[:, :, 0, :].reshape(B, 2, 8, 128)).sum(axis=(1, 2, 3))
    lag_sum = lag_acc - seam

    # edge corrections per core (both channels folded together)
    e_w = (s64[:, :, :, 0] ** 2).sum(axis=(1, 2)) + \
          (s64[:, :, :, -1] ** 2).sum(axis=(1, 2))
    e_h = (s64[:, :, 0, :] ** 2).sum(axis=(1, 2)) + \
          (s64[:, :, -1, :] ** 2).sum(axis=(1, 2))

    num = (4.0 * s2 - e_w - e_h - 2.0 * lag_sum).sum()
    cnt = B * 2 * H * (W - 1)

    ncc_loss = -cc_sum / (B * H * W)
    smooth = 0.5 * num / cnt * ALPHA
    total = ncc_loss + smooth
    return np.array([total, ncc_loss, smooth], dtype=np.float32)
